# revision 3
# baseline (speedup 1.0000x reference)
"""Trainium2 Bass kernel for DenseGatPerfPlayerModel — v2 (bf16 + PE tiling).

Design (8 NeuronCores, 4 graphs/core, data-parallel over B):
  - ALL fat matmuls in bf16 (fp32 moving operand costs 4 cyc/row; bf16 1).
  - Heads split in two halves (A: 0-3, B: 4-7), head j of a half at partition
    base 32j.  Score matmuls are K=16 row-tiles (tile_position=(32j,0)) so up
    to 4 heads stream concurrently through distinct 32-row strips of the PE.
  - o matmuls are M=32 col-tiles (tile_position=(0,32j)): 4 heads of a half
    accumulate into ONE psum bank at 32-partition stride.  Softmax denominator
    comes for free from a ones-column folded into the Wv weights (bias-row
    trick): o row 32j+16 = sum_m p[m,n].
  - Projections (x0/q/k/v/k2/v2/...) batched across all 4 graphs (free dim
    2048) with weights as lhsT -> one LDWEIGHTS amortized over 4 matmuls.
  - exp on ScalarE (only engine with exp), psum->sbuf bf16; adjacency mask
    multiply split between VectorE and GpSimd (bf16 2x mode on DVE).
  - Layer 2 evaluated only at the query node via block-diagonal tricks:
    s2[m,h] in ONE matmul per (g,chunk) (lhsT=k2t chunk [128(he),128(m)],
    rhs=block-diag q2), o2[(h,e),h'] via 16-stride v2, diagonal extracted with
    a mask+reduce.  den2 = ones^T pm2 matmul.
  - PSUM budget (8 banks): scores [128,2,512]x2bufs = 4, o [128,512]x2 = 2,
    misc [128,512]x2 = 2.
"""

import numpy as np

B, N = 32, 512
G = 4
NCORES = 8
H, DH, DO, DLIN = 8, 16, 16, 64
DIN, DINIT = 16, 64
SCALE = 1999853.335557038
P = 128
MC = N // P


def _build_nc(debug=False):
    from contextlib import ExitStack

    import concourse.mybir as mybir
    import concourse.tile as tile
    from concourse import bacc

    f32 = mybir.dt.float32
    bf = mybir.dt.bfloat16
    AF = mybir.ActivationFunctionType
    ALU = mybir.AluOpType

    nc = bacc.Bacc()

    # ---- DRAM parameters (per-core shard) ----
    adjb_d = nc.declare_dram_parameter("adjb", [G, MC, P, N], bf, isOutput=False)
    nf_d = nc.declare_dram_parameter("nf", [DIN + 1, G * N], bf, isOutput=False)
    adjq_d = nc.declare_dram_parameter("adjq", [G, P, MC], bf, isOutput=False)
    oh_d = nc.declare_dram_parameter("oh", [G, P, MC], f32, isOutput=False)

    wspec_bf = {
        "Wi": [DIN + 1, DINIT],
        "WqA": [DINIT + 1, P], "WqB": [DINIT + 1, P],
        "WkA": [DINIT + 1, P], "WkB": [DINIT + 1, P],
        "WvAB": [DINIT + 1, 256],
        "WlA": [P, DLIN], "WlB": [P, DLIN],
        "SelA": [P, H], "SelB": [P, H],
        "EA": [H, P], "EB": [H, P],
        "Wk2": [DLIN + 1, P], "Wv2I": [DLIN + 1, P + DLIN],
        "bm16": [P, H], "ones": [P, 1],
    }
    wspec_f32 = {
        "Wq2": [DLIN + 1, P], "Wl2": [P, DLIN], "E2": [H, P],
        "Wf0": [2 * DLIN, 128], "Wf1": [128, 64], "Wf2": [64, 1],
        "bl0": [DLIN, 1], "bl1": [DLIN, 1],
        "bi": [DINIT, 1],
        "bf0": [128, 1], "bf1": [64, 1], "bf2": [1, 1],
    }
    w_d = {k: nc.declare_dram_parameter(k, s, bf, isOutput=False)
           for k, s in wspec_bf.items()}
    w_d.update({k: nc.declare_dram_parameter(k, s, f32, isOutput=False)
                for k, s in wspec_f32.items()})
    out_d = nc.declare_dram_parameter("out", [1, G], f32, isOutput=True)
    if debug:
        dbg_d = {
            "d_x0": nc.declare_dram_parameter("d_x0", [DINIT + 1, G * N], bf, isOutput=True),
            "d_qtA": nc.declare_dram_parameter("d_qtA", [P, G * N], bf, isOutput=True),
            "d_ktA": nc.declare_dram_parameter("d_ktA", [P, G * N], bf, isOutput=True),
            "d_osbA": nc.declare_dram_parameter("d_osbA", [P, N], bf, isOutput=True),
            "d_x1": nc.declare_dram_parameter("d_x1", [DLIN + 1, G * N], bf, isOutput=True),
            "d_feat": nc.declare_dram_parameter("d_feat", [2 * DLIN, G], f32, isOutput=True),
        }

    with tile.TileContext(nc) as tc, ExitStack() as ctx:
        wpool = ctx.enter_context(tc.tile_pool(name="w", bufs=1))
        data = ctx.enter_context(tc.tile_pool(name="data", bufs=1))
        work = ctx.enter_context(tc.tile_pool(name="work", bufs=3))
        pmpool = ctx.enter_context(tc.tile_pool(name="pm", bufs=1))
        osb = ctx.enter_context(tc.tile_pool(name="osb", bufs=2))
        ps = ctx.enter_context(tc.tile_pool(name="ps", bufs=2, space="PSUM"))

        # ---- weights + inputs to SBUF (critical-path first, 3 queues) ----
        dma_engines = [nc.sync, nc.gpsimd]
        dma_ctr = [0]

        def dma(dst, src):
            eng = dma_engines[dma_ctr[0] % len(dma_engines)]
            dma_ctr[0] += 1
            eng.dma_start(dst, src)

        W = {}
        for k in list(wspec_bf) + list(wspec_f32):
            dt = bf if k in wspec_bf else f32
            shape = wspec_bf.get(k) or wspec_f32[k]
            W[k] = wpool.tile(shape, dt, tag=f"w_{k}", name=f"w_{k}")
        nf = data.tile([DIN + 1, G * N], bf, tag="nf")
        adjb = data.tile([P, G * MC, N], bf, tag="adjb")
        adjq_sb = wpool.tile([P, G, MC], bf, tag="adjq")
        oh_sb = wpool.tile([P, G, MC], f32, tag="oh")
        # phase-A critical path first
        dma(nf[:], nf_d[:])
        for k in ("Wi", "bi", "WqA", "WkA", "WqB", "WkB", "WvAB"):
            dma(W[k][:], w_d[k][:])
        for g in range(G):
            for mc in range(MC):
                dma(adjb[:, g * MC + mc, :], adjb_d[g, mc])
        for k in list(wspec_bf) + list(wspec_f32):
            if k in ("Wi", "bi", "WqA", "WkA", "WqB", "WkB", "WvAB"):
                continue
            dma(W[k][:], w_d[k][:])
        for g in range(G):
            dma(adjq_sb[:, g, :], adjq_d[g])
            dma(oh_sb[:, g, :], oh_d[g])

        # persistent SBUF state
        x0 = data.tile([DINIT + 1, G * N], bf, tag="x0")
        x1 = data.tile([DLIN + 1, G * N], bf, tag="x1")
        qtA = data.tile([P, G * N], bf, tag="qtA")
        qtB = data.tile([P, G * N], bf, tag="qtB")
        ktA = data.tile([P, G * N], bf, tag="ktA")
        ktB = data.tile([P, G * N], bf, tag="ktB")
        vsb = data.tile([P, G * MC, 256], bf, tag="vsb")  # A cols 0:128, B 128:256
        k2t = data.tile([P, G * N], bf, tag="k2t")
        v2sb = data.tile([P, G * MC, P], bf, tag="v2sb")
        x1nd = data.tile([P, G * MC, DLIN], f32, tag="x1nd")
        x1qa = data.tile([DLIN + 1, G], f32, tag="x1qa")
        q2bd = data.tile([P, G, H], bf, tag="q2bd")
        feat = data.tile([2 * DLIN, G], f32, tag="feat")
        out_sb = data.tile([1, G], f32, tag="out_sb")
        zeros = data.tile([P, 1], bf, tag="zeros")
        nc.vector.memset(zeros[:], 0.0)
        nc.vector.memset(x0[DINIT:DINIT + 1, :], 1.0)
        nc.vector.memset(x1[DLIN:DLIN + 1, :], 1.0)
        nc.vector.memset(x1qa[DLIN:DLIN + 1, :], 1.0)

        mask_ctr = [0]

        def mask_engine():
            # split mask multiplies ~3:2 between DVE and GpSimd
            mask_ctr[0] += 1
            return nc.vector if mask_ctr[0] % 3 < 2 else nc.gpsimd

    # --- helpers -------------------------------------------------------
        def elu(dst, src_ps, bias_ap, p, f, tg):
            # dst(bf16) = elu(src_ps + bias); src_ps is PSUM [p, f]
            e = work.tile([p, f], f32, tag=f"elu_e_{tg}", name=f"elu_e_{tg}", bufs=1)
            nc.scalar.activation(e[:], src_ps, AF.Exp, bias=bias_ap)
            e2 = work.tile([p, f], f32, tag=f"elu_e2_{tg}", name=f"elu_e2_{tg}", bufs=1)
            nc.vector.tensor_scalar(e2[:], e[:], 1.0, 0.0, ALU.subtract, ALU.min)
            r = work.tile([p, f], f32, tag=f"elu_r_{tg}", name=f"elu_r_{tg}", bufs=1)
            nc.scalar.activation(r[:], src_ps, AF.Relu, bias=bias_ap)
            nc.vector.tensor_add(dst, e2[:], r[:])

        # ---- phase A: x0 = elu(nf @ Wi + bi), then q/k/v projections ----
        for t in range(2):
            sp = ps.tile([P, 2, N], f32, tag="s", name="x0ps")
            for r in range(2):
                c = (2 * t + r) * N
                nc.tensor.matmul(sp[0:DINIT, r, :], W["Wi"][:], nf[:, c:c + N],
                                 start=True, stop=True)
            elu(x0[0:DINIT, 2 * t * N:(2 * t + 2) * N],
                sp[0:DINIT, :, :].rearrange("p a n -> p (a n)"),
                W["bi"][:], DINIT, 2 * N, "x0")

        for t in range(2):
            for wn, dst in (("WqA", qtA), ("WkA", ktA), ("WqB", qtB), ("WkB", ktB)):
                sp = ps.tile([P, 2, N], f32, tag="s", name=f"p_{wn}{t}")
                for r in range(2):
                    c = (2 * t + r) * N
                    nc.tensor.matmul(sp[:, r, :], W[wn][:], x0[:, c:c + N],
                                     start=True, stop=True)
                nc.vector.tensor_copy(
                    dst[:, 2 * t * N:(2 * t + 2) * N],
                    sp.rearrange("p a n -> p (a n)"))

        def emit_v():
            for g in range(G):
                for mc in range(MC):
                    vp = ps.tile([P, 256], f32, tag="m", name="vp")
                    c = g * N + mc * P
                    nc.tensor.matmul(vp[:], x0[:, c:c + P], W["WvAB"][:],
                                     start=True, stop=True)
                    nc.vector.tensor_copy(vsb[:, g * MC + mc, :], vp[:])

        # ---- phase B+C: software-pipelined over graphs ----
        pm_refs = {}
        o_ps_refs = {}

        def emit_scores(g, filler=None):
            gc = g * N
            pm_ref = {}
            for half, (qt, kt) in enumerate(((qtA, ktA), (qtB, ktB))):
                for mc in range(MC):
                    for pair in range(2):
                        sp = ps.tile([P, 2, N], f32, tag="s", name="sp")
                        for r in range(2):
                            j = 2 * pair + r
                            nc.tensor.matmul(
                                sp[:, r, :],
                                kt[32 * j:32 * j + DH, gc + mc * P:gc + (mc + 1) * P],
                                qt[32 * j:32 * j + DH, gc:gc + N],
                                start=True, stop=True, tile_position=(32 * j, 0))
                        ex = work.tile([P, 2, N], bf, tag="ex")
                        nc.scalar.activation(ex[:], sp[:], AF.Exp)
                        pmt = pmpool.tile([P, 2, N], bf,
                                          tag=f"pm{g % 2}_{half}_{mc}_{pair}",
                                          name=f"pm{g}_{half}_{mc}_{pair}")
                        mask_engine().tensor_tensor(
                            pmt[:], ex[:],
                            adjb[:, g * MC + mc, None, :].to_broadcast((P, 2, N)),
                            ALU.mult)
                        pm_ref[(half, mc, pair)] = pmt
                        if filler is not None:
                            next(filler, None)
            pm_refs[g] = pm_ref

        def o_gen(g):
            pm_ref = pm_refs.pop(g)
            o_psA = ps.tile([P, N], f32, tag="o", name="opsA")
            o_psB = ps.tile([P, N], f32, tag="o", name="opsB")
            o_ps_refs[g] = (o_psA, o_psB)
            for ja in range(4):
                jb = (ja + 1) % 4
                for mc in range(MC):
                    for half, o_ps, j in ((0, o_psA, ja), (1, o_psB, jb)):
                        nc.tensor.matmul(
                            o_ps[32 * j:32 * j + 32, :],
                            vsb[:, g * MC + mc, P * half + 32 * j:P * half + 32 * j + 32],
                            pm_ref[(half, mc, j // 2)][:, j % 2, :],
                            start=(mc == 0), stop=(mc == MC - 1),
                            tile_position=(0, 32 * j))
                    yield

        def finish1_tail(g):
            gc = g * N
            o_psA, o_psB = o_ps_refs.pop(g)
            o_sb_ref = {}
            for half, o_ps in ((0, o_psA), (1, o_psB)):
                o_sbt = osb.tile([P, N], bf, tag=f"osb{half}", name=f"osb{half}")
                nc.vector.tensor_copy(o_sbt[:], o_ps[:])
                o_sb_ref[half] = o_sbt

            # normalize + Wl + elu -> x1
            den_ps = ps.tile([H, N], f32, tag="m", name="den_ps")
            nc.tensor.matmul(den_ps[:], W["SelA"][:], o_sb_ref[0][:],
                             start=True, stop=False)
            nc.tensor.matmul(den_ps[:], W["SelB"][:], o_sb_ref[1][:],
                             start=False, stop=True)
            recf = work.tile([H, N], f32, tag="recf")
            nc.vector.reciprocal_approx_fast(recf[:], den_ps[:])
            den_sb = work.tile([H, N], bf, tag="den_sb")
            nc.vector.tensor_copy(den_sb[:], recf[:])
            scr_ref = {}
            for half in range(2):
                db_ps = ps.tile([P, N], f32, tag="m", name="db_ps")
                nc.tensor.matmul(db_ps[:], W["EA" if half == 0 else "EB"][:],
                                 den_sb[:], start=True, stop=True)
                scr = work.tile([P, N], bf, tag=f"scr{half}", name=f"scr{half}",
                                bufs=2)
                nc.vector.scalar_tensor_tensor(
                    scr[:], o_sb_ref[half][:], 1.0, db_ps[:], ALU.mult, ALU.mult)
                scr_ref[half] = scr
            x1_ps = ps.tile([DLIN, N], f32, tag="m", name="x1_ps")
            nc.tensor.matmul(x1_ps[:], W["WlA"][:], scr_ref[0][:],
                             start=True, stop=False)
            nc.tensor.matmul(x1_ps[:], W["WlB"][:], scr_ref[1][:],
                             start=False, stop=True)
            elu(x1[0:DLIN, gc:gc + N], x1_ps[:], W["bl0"][:], DLIN, N, "x1")

        def emit_finish2(g):
            gc = g * N
            # ---- layer 2 for this graph ----
            # k2t / v2 / x1nd chunks
            kp = ps.tile([P, N], f32, tag="m", name="k2p")
            nc.tensor.matmul(kp[:], W["Wk2"][:], x1[:, gc:gc + N],
                             start=True, stop=True)
            nc.vector.tensor_copy(k2t[:, gc:gc + N], kp[:])
            for mc in range(MC):
                c = gc + mc * P
                vp2 = ps.tile([P, P + DLIN], f32, tag="m", name="v2p")
                nc.tensor.matmul(vp2[:], x1[:, c:c + P], W["Wv2I"][:],
                                 start=True, stop=True)
                nc.vector.tensor_copy(v2sb[:, g * MC + mc, :], vp2[:, 0:P])
                nc.vector.tensor_copy(x1nd[:, g * MC + mc, :], vp2[:, P:P + DLIN])
            # x1 at query node
            xq_ps = ps.tile([DLIN, MC], f32, tag="m", name="xq_ps")
            for mc in range(MC):
                nc.tensor.matmul(xq_ps[:, mc:mc + 1], x1nd[:, g * MC + mc, :],
                                 oh_sb[:, g, mc:mc + 1], start=True, stop=True)
            xq = work.tile([DLIN, 1], f32, tag="xq")
            nc.vector.tensor_reduce(xq[:], xq_ps[:], mybir.AxisListType.X,
                                    ALU.add)
            nc.vector.tensor_copy(feat[0:DLIN, g:g + 1], xq[:])
            nc.gpsimd.tensor_copy(x1qa[0:DLIN, g:g + 1], xq[:])
            # q2 block-diag
            q2_ps = ps.tile([P, 1], f32, tag="m", name="q2_ps")
            nc.tensor.matmul(q2_ps[:], W["Wq2"][:], x1qa[:, g:g + 1],
                             start=True, stop=True)
            nc.vector.scalar_tensor_tensor(
                q2bd[:, g, :], q2_ps[:, 0:1].to_broadcast((P, H)), 1.0,
                W["bm16"][:], ALU.mult, ALU.mult)
            # attention at query node: s2/o2/den2 (mc in free dim, no groups)
            o2_ps = ps.tile([P, MC, H], f32, tag="m", name="o2_ps")
            d2_ps = ps.tile([H, MC], f32, tag="m", name="d2_ps")
            for mc in range(MC):
                c = gc + mc * P
                s2_ps = ps.tile([P, 2, N], f32, tag="s", name="s2_ps")
                nc.tensor.matmul(s2_ps[:, 0, 0:H], k2t[:, c:c + P],
                                 q2bd[:, g, :], start=True, stop=True)
                ex2 = work.tile([P, H], bf, tag="ex2")
                nc.scalar.activation(ex2[:], s2_ps[:, 0, 0:H], AF.Exp)
                pm2 = work.tile([P, H], bf, tag="pm2")
                nc.vector.tensor_tensor(
                    pm2[:], ex2[:],
                    adjq_sb[:, g, mc:mc + 1].to_broadcast((P, H)), ALU.mult)
                nc.tensor.matmul(o2_ps[:, mc, :], v2sb[:, g * MC + mc, :],
                                 pm2[:], start=True, stop=True)
                nc.tensor.matmul(d2_ps[:, mc:mc + 1], pm2[:], W["ones"][:],
                                 start=True, stop=True)
            # extract diag blocks + denominators
            o2m = work.tile([P, MC, H], f32, tag="o2m")
            nc.vector.tensor_tensor(
                o2m[:], o2_ps[:],
                W["bm16"][:, None, :].to_broadcast((P, MC, H)), ALU.mult)
            o2v = work.tile([P, 1], f32, tag="o2v")
            nc.vector.tensor_reduce(o2v[:], o2m[:], mybir.AxisListType.XY,
                                    ALU.add)
            den2 = work.tile([H, 1], f32, tag="den2")
            nc.vector.tensor_reduce(den2[:], d2_ps[:], mybir.AxisListType.X,
                                    ALU.add)
            rec2 = work.tile([H, 1], f32, tag="rec2")
            nc.vector.reciprocal(rec2[:], den2[:])
            scb_ps = ps.tile([P, 1], f32, tag="m", name="scb_ps")
            nc.tensor.matmul(scb_ps[:], W["E2"][:], rec2[:],
                             start=True, stop=True)
            scr2 = work.tile([P, 1], f32, tag="scr2")
            nc.vector.scalar_tensor_tensor(scr2[:], scb_ps[:], 1.0, o2v[:],
                                           ALU.mult, ALU.mult)
            x2_ps = ps.tile([DLIN, 1], f32, tag="m", name="x2_ps")
            nc.tensor.matmul(x2_ps[:], W["Wl2"][:], scr2[:],
                             start=True, stop=True)
            elu(feat[DLIN:2 * DLIN, g:g + 1], x2_ps[:], W["bl1"][:],
                DLIN, 1, "x2")

        emit_scores(0)
        emit_v()
        gen = o_gen(0)
        emit_scores(1, gen)
        for _ in gen:
            pass
        finish1_tail(0)
        gen = o_gen(1)
        emit_scores(2, gen)
        for _ in gen:
            pass
        finish1_tail(1)
        emit_finish2(0)
        gen = o_gen(2)
        emit_scores(3, gen)
        for _ in gen:
            pass
        finish1_tail(2)
        emit_finish2(1)
        gen = o_gen(3)
        for _ in range(8):
            next(gen, None)
        emit_finish2(2)
        for _ in gen:
            pass
        finish1_tail(3)
        emit_finish2(3)

        # ---- MLP head over all graphs ----
        h1_ps = ps.tile([128, G], f32, tag="m", name="h1_ps")
        nc.tensor.matmul(h1_ps[:], W["Wf0"][:], feat[:], start=True, stop=True)
        h1 = work.tile([128, G], f32, tag="h1")
        elu(h1[:], h1_ps[:], W["bf0"][:], 128, G, "m1")
        h2_ps = ps.tile([64, G], f32, tag="m", name="h2_ps")
        nc.tensor.matmul(h2_ps[:], W["Wf1"][:], h1[:], start=True, stop=True)
        h2 = work.tile([64, G], f32, tag="h2")
        elu(h2[:], h2_ps[:], W["bf1"][:], 64, G, "m2")
        h3_ps = ps.tile([1, G], f32, tag="m", name="h3_ps")
        nc.tensor.matmul(h3_ps[:], W["Wf2"][:], h2[:], start=True, stop=True)
        if debug:
            nc.sync.dma_start(dbg_d["d_x0"][:], x0[:])
            nc.sync.dma_start(dbg_d["d_qtA"][:], qtA[:])
            nc.sync.dma_start(dbg_d["d_ktA"][:], ktA[:])
            nc.sync.dma_start(dbg_d["d_x1"][:], x1[:])
            nc.sync.dma_start(dbg_d["d_feat"][:], feat[:])
        fout = work.tile([1, G], f32, tag="fout")
        # elu in f32 for the final scaled output
        e = work.tile([1, G], f32, tag="fin_e")
        nc.scalar.activation(e[:], h3_ps[:], AF.Exp, bias=W["bf2"][:])
        nc.vector.tensor_scalar(e[:], e[:], 1.0, 0.0, ALU.subtract, ALU.min)
        r = work.tile([1, G], f32, tag="fin_r")
        nc.scalar.activation(r[:], h3_ps[:], AF.Relu, bias=W["bf2"][:])
        nc.vector.tensor_add(fout[:], e[:], r[:])
        nc.vector.tensor_scalar_mul(out_sb[:], fout[:], float(SCALE))
        nc.sync.dma_start(out_d[:], out_sb[:])

    nc.compile()
    return nc


# ======================= host-side marshaling =======================

def _bf16(x):
    import ml_dtypes
    return np.asarray(x, dtype=np.float32).astype(ml_dtypes.bfloat16)


def _prep_weights(inputs):
    f32 = np.float32
    w = {}

    def headcols(Wm, bv, stride, scale=1.0, heads=range(H), ones_col=False):
        # [din+1, 128] with head h (enumerated j) at column stride*j
        din = Wm.shape[0]
        O = np.zeros((din + 1, P), f32)
        for j, h in enumerate(heads):
            O[0:din, stride * j:stride * j + Wm.shape[2]] = Wm[:, h, :] * scale
            O[din, stride * j:stride * j + Wm.shape[2]] = bv[h, :] * scale
            if ones_col:
                O[din, stride * j + DO] = 1.0
        return O

    Wq0 = np.asarray(inputs["Wq0"], f32); bq0 = np.asarray(inputs["bq0"], f32)
    Wk0 = np.asarray(inputs["Wk0"], f32); bk0 = np.asarray(inputs["bk0"], f32)
    Wv0 = np.asarray(inputs["Wv0"], f32); bv0 = np.asarray(inputs["bv0"], f32)
    s = 1.0 / np.sqrt(DH)
    w["WqA"] = headcols(Wq0, bq0, 32, heads=range(0, 4))
    w["WqB"] = headcols(Wq0, bq0, 32, heads=range(4, 8))
    w["WkA"] = headcols(Wk0, bk0, 32, scale=s, heads=range(0, 4))
    w["WkB"] = headcols(Wk0, bk0, 32, scale=s, heads=range(4, 8))
    w["WvAB"] = np.concatenate(
        [headcols(Wv0, bv0, 32, heads=range(0, 4), ones_col=True),
         headcols(Wv0, bv0, 32, heads=range(4, 8), ones_col=True)], axis=1)

    Wl0 = np.asarray(inputs["Wl0"], f32)  # [H*DO, DLIN]
    for half, nm in ((0, "WlA"), (1, "WlB")):
        O = np.zeros((P, DLIN), f32)
        for j in range(4):
            h = 4 * half + j
            O[32 * j:32 * j + DO, :] = Wl0[DO * h:DO * (h + 1), :]
        w[nm] = O
    for half, nm in ((0, "SelA"), (1, "SelB")):
        O = np.zeros((P, H), f32)
        for j in range(4):
            O[32 * j + DO, 4 * half + j] = 1.0
        w[nm] = O
    for half, nm in ((0, "EA"), (1, "EB")):
        O = np.zeros((H, P), f32)
        for j in range(4):
            O[4 * half + j, 32 * j:32 * j + DO] = 1.0
        w[nm] = O

    Wq1 = np.asarray(inputs["Wq1"], f32); bq1 = np.asarray(inputs["bq1"], f32)
    Wk1 = np.asarray(inputs["Wk1"], f32); bk1 = np.asarray(inputs["bk1"], f32)
    Wv1 = np.asarray(inputs["Wv1"], f32); bv1 = np.asarray(inputs["bv1"], f32)
    w["Wq2"] = headcols(Wq1, bq1, DH)
    w["Wk2"] = headcols(Wk1, bk1, DH, scale=s)
    Wv2 = headcols(Wv1, bv1, DH)
    I64a = np.concatenate([np.eye(DLIN, dtype=f32),
                           np.zeros((1, DLIN), f32)], axis=0)
    w["Wv2I"] = np.concatenate([Wv2, I64a], axis=1)
    Wl1 = np.asarray(inputs["Wl1"], f32)
    O = np.zeros((P, DLIN), f32)
    for h in range(H):
        O[DH * h:DH * h + DO, :] = Wl1[DO * h:DO * (h + 1), :]
    w["Wl2"] = O
    E2 = np.zeros((H, P), f32)
    for h in range(H):
        E2[h, DH * h:DH * h + DO] = 1.0
    w["E2"] = E2
    bm = np.zeros((P, H), f32)
    for h in range(H):
        bm[DH * h:DH * h + DH, h] = 1.0
    w["bm16"] = bm
    w["ones"] = np.ones((P, 1), f32)
    w["Wi"] = np.concatenate([np.asarray(inputs["W_init"], f32),
                              np.zeros((1, DINIT), f32)], axis=0)
    w["Wi"][DIN, :] = 0.0  # bias folded separately via bi (ACT bias)
    w["Wf0"] = np.asarray(inputs["Wf0"], f32)
    w["Wf1"] = np.asarray(inputs["Wf1"], f32)
    w["Wf2"] = np.asarray(inputs["Wf2"], f32)

    F32W = ("Wq2", "Wl2", "E2", "Wf0", "Wf1", "Wf2")
    out = {k: (np.asarray(v, np.float32) if k in F32W else _bf16(v))
           for k, v in w.items()}
    out["bi"] = np.asarray(inputs["b_init"], np.float32).reshape(DINIT, 1)
    out["bl0"] = np.asarray(inputs["bl0"], np.float32).reshape(DLIN, 1)
    out["bl1"] = np.asarray(inputs["bl1"], np.float32).reshape(DLIN, 1)
    out["bf0"] = np.asarray(inputs["bf0"], np.float32).reshape(128, 1)
    out["bf1"] = np.asarray(inputs["bf1"], np.float32).reshape(64, 1)
    out["bf2"] = np.asarray(inputs["bf2"], np.float32).reshape(1, 1)
    return out


def _prep_core_inputs(inputs, core):
    f32 = np.float32
    sl = slice(core * G, (core + 1) * G)
    nfi = np.asarray(inputs["node_features"], f32)[sl]     # [G, N, DIN]
    adj = np.asarray(inputs["adj"], f32)[sl]               # [G, N, N]
    masks = np.asarray(inputs["masks"], f32)[sl]           # [G, N]
    qidx = np.asarray(inputs["query_idxs"])[sl]            # [G]

    # nf: [DIN+1, G*N]; row DIN = 1 (augmentation for bias rows of W*)
    nf = np.concatenate([np.transpose(nfi, (0, 2, 1)),
                         np.ones((G, 1, N), f32)], axis=1)   # [G, 17, N]
    nf = np.transpose(nf, (1, 0, 2)).reshape(DIN + 1, G * N)

    # adjb[g, mc, p, n] = keymask(m = mc*128+p source, n dest)
    adjT = ((np.transpose(adj, (0, 2, 1)) > 0) & (masks[:, :, None] > 0))
    adjb = adjT.astype(f32).reshape(G, MC, P, N)

    adjq = np.stack([(adj[g, qidx[g]] > 0) & (masks[g] > 0) for g in range(G)])
    adjq = adjq.astype(f32).reshape(G, MC, P).transpose(0, 2, 1)  # [G, P, MC]
    onehot = np.zeros((G, N), f32)
    onehot[np.arange(G), qidx] = 1.0
    onehot = onehot.reshape(G, MC, P).transpose(0, 2, 1)

    return {
        "nf": _bf16(nf),
        "adjb": _bf16(adjb),
        "adjq": _bf16(adjq),
        "oh": np.asarray(onehot, np.float32),
    }


def kernel(**inputs) -> np.ndarray:
    from concourse.bass_utils import run_bass_kernel_spmd

    nc = _build_nc()
    w = _prep_weights(inputs)
    in_maps = []
    for core in range(NCORES):
        m = _prep_core_inputs(inputs, core)
        m.update(w)
        in_maps.append(m)
    res = run_bass_kernel_spmd(nc, in_maps, list(range(NCORES)))
    out = np.concatenate([res.results[i]["out"][0] for i in range(NCORES)])
    return out.astype(np.float32).reshape(B, 1)


# revision 4
# speedup vs baseline: 1.0088x; 1.0088x over previous
"""Trainium2 Bass kernel for DenseGatPerfPlayerModel — v2 (bf16 + PE tiling).

Design (8 NeuronCores, 4 graphs/core, data-parallel over B):
  - ALL fat matmuls in bf16 (fp32 moving operand costs 4 cyc/row; bf16 1).
  - Heads split in two halves (A: 0-3, B: 4-7), head j of a half at partition
    base 32j.  Score matmuls are K=16 row-tiles (tile_position=(32j,0)) so up
    to 4 heads stream concurrently through distinct 32-row strips of the PE.
  - o matmuls are M=32 col-tiles (tile_position=(0,32j)): 4 heads of a half
    accumulate into ONE psum bank at 32-partition stride.  Softmax denominator
    comes for free from a ones-column folded into the Wv weights (bias-row
    trick): o row 32j+16 = sum_m p[m,n].
  - Projections (x0/q/k/v/k2/v2/...) batched across all 4 graphs (free dim
    2048) with weights as lhsT -> one LDWEIGHTS amortized over 4 matmuls.
  - exp on ScalarE (only engine with exp), psum->sbuf bf16; adjacency mask
    multiply split between VectorE and GpSimd (bf16 2x mode on DVE).
  - Layer 2 evaluated only at the query node via block-diagonal tricks:
    s2[m,h] in ONE matmul per (g,chunk) (lhsT=k2t chunk [128(he),128(m)],
    rhs=block-diag q2), o2[(h,e),h'] via 16-stride v2, diagonal extracted with
    a mask+reduce.  den2 = ones^T pm2 matmul.
  - PSUM budget (8 banks): scores [128,2,512]x2bufs = 4, o [128,512]x2 = 2,
    misc [128,512]x2 = 2.
"""

import numpy as np

B, N = 32, 512
G = 4
NCORES = 8
H, DH, DO, DLIN = 8, 16, 16, 64
DIN, DINIT = 16, 64
SCALE = 1999853.335557038
P = 128
MC = N // P


def _build_nc(debug=False):
    from contextlib import ExitStack

    import concourse.mybir as mybir
    import concourse.tile as tile
    from concourse import bacc

    f32 = mybir.dt.float32
    bf = mybir.dt.bfloat16
    AF = mybir.ActivationFunctionType
    ALU = mybir.AluOpType

    nc = bacc.Bacc()

    # ---- DRAM parameters (per-core shard) ----
    adjb_d = nc.declare_dram_parameter("adjb", [G, MC, P, N], bf, isOutput=False)
    nf_d = nc.declare_dram_parameter("nf", [DIN + 1, G * N], bf, isOutput=False)
    adjq_d = nc.declare_dram_parameter("adjq", [G, P, MC], bf, isOutput=False)
    oh_d = nc.declare_dram_parameter("oh", [G, P, MC], f32, isOutput=False)

    wspec_bf = {
        "Wi": [DIN + 1, DINIT],
        "WqA": [DINIT + 1, P], "WqB": [DINIT + 1, P],
        "WkA": [DINIT + 1, P], "WkB": [DINIT + 1, P],
        "WvAB": [DINIT + 1, 256],
        "WlA": [P, DLIN], "WlB": [P, DLIN],
        "SelA": [P, H], "SelB": [P, H],
        "EA": [H, P], "EB": [H, P],
        "Wk2": [DLIN + 1, P], "Wv2I": [DLIN + 1, P + DLIN],
        "bm16": [P, H], "ones": [P, 1],
    }
    wspec_f32 = {
        "Wq2": [DLIN + 1, P], "Wl2": [P, DLIN], "E2": [H, P],
        "Wf0": [2 * DLIN, 128], "Wf1": [128, 64], "Wf2": [64, 1],
        "bl0": [DLIN, 1], "bl1": [DLIN, 1],
        "bi": [DINIT, 1],
        "bf0": [128, 1], "bf1": [64, 1], "bf2": [1, 1],
    }
    w_d = {k: nc.declare_dram_parameter(k, s, bf, isOutput=False)
           for k, s in wspec_bf.items()}
    w_d.update({k: nc.declare_dram_parameter(k, s, f32, isOutput=False)
                for k, s in wspec_f32.items()})
    out_d = nc.declare_dram_parameter("out", [1, G], f32, isOutput=True)
    if debug:
        dbg_d = {
            "d_x0": nc.declare_dram_parameter("d_x0", [DINIT + 1, G * N], bf, isOutput=True),
            "d_qtA": nc.declare_dram_parameter("d_qtA", [P, G * N], bf, isOutput=True),
            "d_ktA": nc.declare_dram_parameter("d_ktA", [P, G * N], bf, isOutput=True),
            "d_osbA": nc.declare_dram_parameter("d_osbA", [P, N], bf, isOutput=True),
            "d_x1": nc.declare_dram_parameter("d_x1", [DLIN + 1, G * N], bf, isOutput=True),
            "d_feat": nc.declare_dram_parameter("d_feat", [2 * DLIN, G], f32, isOutput=True),
        }

    with tile.TileContext(nc) as tc, ExitStack() as ctx:
        wpool = ctx.enter_context(tc.tile_pool(name="w", bufs=1))
        data = ctx.enter_context(tc.tile_pool(name="data", bufs=1))
        work = ctx.enter_context(tc.tile_pool(name="work", bufs=3))
        pmpool = ctx.enter_context(tc.tile_pool(name="pm", bufs=1))
        osb = ctx.enter_context(tc.tile_pool(name="osb", bufs=2))
        ps = ctx.enter_context(tc.tile_pool(name="ps", bufs=2, space="PSUM"))

        # ---- weights + inputs to SBUF (critical-path first, 3 queues) ----
        dma_engines = [nc.sync, nc.gpsimd]
        dma_ctr = [0]

        def dma(dst, src):
            eng = dma_engines[dma_ctr[0] % len(dma_engines)]
            dma_ctr[0] += 1
            eng.dma_start(dst, src)

        W = {}
        for k in list(wspec_bf) + list(wspec_f32):
            dt = bf if k in wspec_bf else f32
            shape = wspec_bf.get(k) or wspec_f32[k]
            W[k] = wpool.tile(shape, dt, tag=f"w_{k}", name=f"w_{k}")
        nf = data.tile([DIN + 1, G * N], bf, tag="nf")
        adjb = data.tile([P, G * MC, N], bf, tag="adjb")
        adjq_sb = wpool.tile([P, G, MC], bf, tag="adjq")
        oh_sb = wpool.tile([P, G, MC], f32, tag="oh")
        # phase-A critical path first
        dma(nf[:], nf_d[:])
        for k in ("Wi", "bi", "WqA", "WkA", "WqB", "WkB", "WvAB"):
            dma(W[k][:], w_d[k][:])
        for g in range(G):
            for mc in range(MC):
                dma(adjb[:, g * MC + mc, :], adjb_d[g, mc])
        for k in list(wspec_bf) + list(wspec_f32):
            if k in ("Wi", "bi", "WqA", "WkA", "WqB", "WkB", "WvAB"):
                continue
            dma(W[k][:], w_d[k][:])
        for g in range(G):
            dma(adjq_sb[:, g, :], adjq_d[g])
            dma(oh_sb[:, g, :], oh_d[g])

        # persistent SBUF state
        x0 = data.tile([DINIT + 1, G * N], bf, tag="x0")
        x1 = data.tile([DLIN + 1, G * N], bf, tag="x1")
        qtA = data.tile([P, G * N], bf, tag="qtA")
        qtB = data.tile([P, G * N], bf, tag="qtB")
        ktA = data.tile([P, G * N], bf, tag="ktA")
        ktB = data.tile([P, G * N], bf, tag="ktB")
        vsb = data.tile([P, G * MC, 256], bf, tag="vsb")  # A cols 0:128, B 128:256
        k2t = data.tile([P, G * N], bf, tag="k2t")
        v2sb = data.tile([P, G * MC, P], bf, tag="v2sb")
        x1nd = data.tile([P, G * MC, DLIN], f32, tag="x1nd")
        x1qa = data.tile([DLIN + 1, G], f32, tag="x1qa")
        q2bd = data.tile([P, G, H], bf, tag="q2bd")
        feat = data.tile([2 * DLIN, G], f32, tag="feat")
        out_sb = data.tile([1, G], f32, tag="out_sb")
        zeros = data.tile([P, 1], bf, tag="zeros")
        nc.vector.memset(zeros[:], 0.0)
        nc.vector.memset(x0[DINIT:DINIT + 1, :], 1.0)
        nc.vector.memset(x1[DLIN:DLIN + 1, :], 1.0)
        nc.vector.memset(x1qa[DLIN:DLIN + 1, :], 1.0)
        warm = data.tile([1, 1], f32, tag="warm")
        nc.scalar.activation(warm[:], zeros[0:1, 0:1], AF.Exp)

        mask_ctr = [0]

        def mask_engine():
            # split mask multiplies ~3:2 between DVE and GpSimd
            mask_ctr[0] += 1
            return nc.vector if mask_ctr[0] % 3 < 2 else nc.gpsimd

    # --- helpers -------------------------------------------------------
        def elu(dst, src_ps, bias_ap, p, f, tg):
            # dst(bf16) = elu(src_ps + bias); src_ps is PSUM [p, f]
            e = work.tile([p, f], f32, tag=f"elu_e_{tg}", name=f"elu_e_{tg}", bufs=1)
            nc.scalar.activation(e[:], src_ps, AF.Exp, bias=bias_ap)
            e2 = work.tile([p, f], f32, tag=f"elu_e2_{tg}", name=f"elu_e2_{tg}", bufs=1)
            nc.vector.tensor_scalar(e2[:], e[:], 1.0, 0.0, ALU.subtract, ALU.min)
            r = work.tile([p, f], f32, tag=f"elu_r_{tg}", name=f"elu_r_{tg}", bufs=1)
            nc.scalar.activation(r[:], src_ps, AF.Relu, bias=bias_ap)
            nc.vector.tensor_add(dst, e2[:], r[:])

        # ---- phase A: x0 = elu(nf @ Wi + bi), then q/k/v projections ----
        for t in range(2):
            sp = ps.tile([P, 2, N], f32, tag="s", name="x0ps")
            for r in range(2):
                c = (2 * t + r) * N
                nc.tensor.matmul(sp[0:DINIT, r, :], W["Wi"][:], nf[:, c:c + N],
                                 start=True, stop=True)
            elu(x0[0:DINIT, 2 * t * N:(2 * t + 2) * N],
                sp[0:DINIT, :, :].rearrange("p a n -> p (a n)"),
                W["bi"][:], DINIT, 2 * N, "x0")

        for t in range(2):
            for wn, dst in (("WqA", qtA), ("WkA", ktA), ("WqB", qtB), ("WkB", ktB)):
                sp = ps.tile([P, 2, N], f32, tag="s", name=f"p_{wn}{t}")
                for r in range(2):
                    c = (2 * t + r) * N
                    nc.tensor.matmul(sp[:, r, :], W[wn][:], x0[:, c:c + N],
                                     start=True, stop=True)
                nc.vector.tensor_copy(
                    dst[:, 2 * t * N:(2 * t + 2) * N],
                    sp.rearrange("p a n -> p (a n)"))

        def emit_v():
            for g in range(G):
                for mc in range(MC):
                    vp = ps.tile([P, 256], f32, tag="m", name="vp")
                    c = g * N + mc * P
                    nc.tensor.matmul(vp[:], x0[:, c:c + P], W["WvAB"][:],
                                     start=True, stop=True)
                    nc.vector.tensor_copy(vsb[:, g * MC + mc, :], vp[:])

        # ---- phase B+C: software-pipelined over graphs ----
        pm_refs = {}
        o_ps_refs = {}

        def emit_scores(g, filler=None):
            gc = g * N
            pm_ref = {}
            for half, (qt, kt) in enumerate(((qtA, ktA), (qtB, ktB))):
                for mc in range(MC):
                    for pair in range(2):
                        sp = ps.tile([P, 2, N], f32, tag="s", name="sp")
                        for r in range(2):
                            j = 2 * pair + r
                            nc.tensor.matmul(
                                sp[:, r, :],
                                kt[32 * j:32 * j + DH, gc + mc * P:gc + (mc + 1) * P],
                                qt[32 * j:32 * j + DH, gc:gc + N],
                                start=True, stop=True, tile_position=(32 * j, 0))
                        ex = work.tile([P, 2, N], bf, tag="ex")
                        nc.scalar.activation(ex[:], sp[:], AF.Exp)
                        pmt = pmpool.tile([P, 2, N], bf,
                                          tag=f"pm{g % 2}_{half}_{mc}_{pair}",
                                          name=f"pm{g}_{half}_{mc}_{pair}")
                        mask_engine().tensor_tensor(
                            pmt[:], ex[:],
                            adjb[:, g * MC + mc, None, :].to_broadcast((P, 2, N)),
                            ALU.mult)
                        pm_ref[(half, mc, pair)] = pmt
                        if filler is not None:
                            next(filler, None)
            pm_refs[g] = pm_ref

        def o_gen(g):
            pm_ref = pm_refs.pop(g)
            o_psA = ps.tile([P, N], f32, tag="o", name="opsA")
            o_psB = ps.tile([P, N], f32, tag="o", name="opsB")
            o_ps_refs[g] = (o_psA, o_psB)
            for ja in range(4):
                jb = (ja + 1) % 4
                for mc in range(MC):
                    for half, o_ps, j in ((0, o_psA, ja), (1, o_psB, jb)):
                        nc.tensor.matmul(
                            o_ps[32 * j:32 * j + 32, :],
                            vsb[:, g * MC + mc, P * half + 32 * j:P * half + 32 * j + 32],
                            pm_ref[(half, mc, j // 2)][:, j % 2, :],
                            start=(mc == 0), stop=(mc == MC - 1),
                            tile_position=(0, 32 * j))
                    yield

        def finish1_tail(g):
            gc = g * N
            o_psA, o_psB = o_ps_refs.pop(g)
            o_sb_ref = {}
            for half, o_ps in ((0, o_psA), (1, o_psB)):
                o_sbt = osb.tile([P, N], bf, tag=f"osb{half}", name=f"osb{half}")
                nc.vector.tensor_copy(o_sbt[:], o_ps[:])
                o_sb_ref[half] = o_sbt

            # normalize + Wl + elu -> x1
            den_ps = ps.tile([H, N], f32, tag="m", name="den_ps")
            nc.tensor.matmul(den_ps[:], W["SelA"][:], o_sb_ref[0][:],
                             start=True, stop=False)
            nc.tensor.matmul(den_ps[:], W["SelB"][:], o_sb_ref[1][:],
                             start=False, stop=True)
            recf = work.tile([H, N], f32, tag="recf")
            nc.vector.reciprocal_approx_fast(recf[:], den_ps[:])
            den_sb = work.tile([H, N], bf, tag="den_sb")
            nc.vector.tensor_copy(den_sb[:], recf[:])
            scr_ref = {}
            for half in range(2):
                db_ps = ps.tile([P, N], f32, tag="m", name="db_ps")
                nc.tensor.matmul(db_ps[:], W["EA" if half == 0 else "EB"][:],
                                 den_sb[:], start=True, stop=True)
                scr = work.tile([P, N], bf, tag=f"scr{half}", name=f"scr{half}",
                                bufs=2)
                nc.vector.scalar_tensor_tensor(
                    scr[:], o_sb_ref[half][:], 1.0, db_ps[:], ALU.mult, ALU.mult)
                scr_ref[half] = scr
            x1_ps = ps.tile([DLIN, N], f32, tag="m", name="x1_ps")
            nc.tensor.matmul(x1_ps[:], W["WlA"][:], scr_ref[0][:],
                             start=True, stop=False)
            nc.tensor.matmul(x1_ps[:], W["WlB"][:], scr_ref[1][:],
                             start=False, stop=True)
            elu(x1[0:DLIN, gc:gc + N], x1_ps[:], W["bl0"][:], DLIN, N, "x1")

        def emit_finish2(g):
            gc = g * N
            # ---- layer 2 for this graph ----
            # k2t / v2 / x1nd chunks
            kp = ps.tile([P, N], f32, tag="m", name="k2p")
            nc.tensor.matmul(kp[:], W["Wk2"][:], x1[:, gc:gc + N],
                             start=True, stop=True)
            nc.vector.tensor_copy(k2t[:, gc:gc + N], kp[:])
            for mc in range(MC):
                c = gc + mc * P
                vp2 = ps.tile([P, P + DLIN], f32, tag="m", name="v2p")
                nc.tensor.matmul(vp2[:], x1[:, c:c + P], W["Wv2I"][:],
                                 start=True, stop=True)
                nc.vector.tensor_copy(v2sb[:, g * MC + mc, :], vp2[:, 0:P])
                nc.vector.tensor_copy(x1nd[:, g * MC + mc, :], vp2[:, P:P + DLIN])
            # x1 at query node
            xq_ps = ps.tile([DLIN, MC], f32, tag="m", name="xq_ps")
            for mc in range(MC):
                nc.tensor.matmul(xq_ps[:, mc:mc + 1], x1nd[:, g * MC + mc, :],
                                 oh_sb[:, g, mc:mc + 1], start=True, stop=True)
            xq = work.tile([DLIN, 1], f32, tag="xq")
            nc.vector.tensor_reduce(xq[:], xq_ps[:], mybir.AxisListType.X,
                                    ALU.add)
            nc.vector.tensor_copy(feat[0:DLIN, g:g + 1], xq[:])
            nc.gpsimd.tensor_copy(x1qa[0:DLIN, g:g + 1], xq[:])
            # q2 block-diag
            q2_ps = ps.tile([P, 1], f32, tag="m", name="q2_ps")
            nc.tensor.matmul(q2_ps[:], W["Wq2"][:], x1qa[:, g:g + 1],
                             start=True, stop=True)
            nc.vector.scalar_tensor_tensor(
                q2bd[:, g, :], q2_ps[:, 0:1].to_broadcast((P, H)), 1.0,
                W["bm16"][:], ALU.mult, ALU.mult)
            # attention at query node: s2/o2/den2 (mc in free dim, no groups)
            o2_ps = ps.tile([P, MC, H], f32, tag="m", name="o2_ps")
            d2_ps = ps.tile([H, MC], f32, tag="m", name="d2_ps")
            for mc in range(MC):
                c = gc + mc * P
                s2_ps = ps.tile([P, 2, N], f32, tag="s", name="s2_ps")
                nc.tensor.matmul(s2_ps[:, 0, 0:H], k2t[:, c:c + P],
                                 q2bd[:, g, :], start=True, stop=True)
                ex2 = work.tile([P, H], bf, tag="ex2")
                nc.scalar.activation(ex2[:], s2_ps[:, 0, 0:H], AF.Exp)
                pm2 = work.tile([P, H], bf, tag="pm2")
                nc.vector.tensor_tensor(
                    pm2[:], ex2[:],
                    adjq_sb[:, g, mc:mc + 1].to_broadcast((P, H)), ALU.mult)
                nc.tensor.matmul(o2_ps[:, mc, :], v2sb[:, g * MC + mc, :],
                                 pm2[:], start=True, stop=True)
                nc.tensor.matmul(d2_ps[:, mc:mc + 1], pm2[:], W["ones"][:],
                                 start=True, stop=True)
            # extract diag blocks + denominators
            o2m = work.tile([P, MC, H], f32, tag="o2m")
            nc.vector.tensor_tensor(
                o2m[:], o2_ps[:],
                W["bm16"][:, None, :].to_broadcast((P, MC, H)), ALU.mult)
            o2v = work.tile([P, 1], f32, tag="o2v")
            nc.vector.tensor_reduce(o2v[:], o2m[:], mybir.AxisListType.XY,
                                    ALU.add)
            den2 = work.tile([H, 1], f32, tag="den2")
            nc.vector.tensor_reduce(den2[:], d2_ps[:], mybir.AxisListType.X,
                                    ALU.add)
            rec2 = work.tile([H, 1], f32, tag="rec2")
            nc.vector.reciprocal(rec2[:], den2[:])
            scb_ps = ps.tile([P, 1], f32, tag="m", name="scb_ps")
            nc.tensor.matmul(scb_ps[:], W["E2"][:], rec2[:],
                             start=True, stop=True)
            scr2 = work.tile([P, 1], f32, tag="scr2")
            nc.vector.scalar_tensor_tensor(scr2[:], scb_ps[:], 1.0, o2v[:],
                                           ALU.mult, ALU.mult)
            x2_ps = ps.tile([DLIN, 1], f32, tag="m", name="x2_ps")
            nc.tensor.matmul(x2_ps[:], W["Wl2"][:], scr2[:],
                             start=True, stop=True)
            elu(feat[DLIN:2 * DLIN, g:g + 1], x2_ps[:], W["bl1"][:],
                DLIN, 1, "x2")

        emit_scores(0)
        emit_v()
        gen = o_gen(0)
        emit_scores(1, gen)
        for _ in gen:
            pass
        finish1_tail(0)
        gen = o_gen(1)
        emit_scores(2, gen)
        for _ in gen:
            pass
        finish1_tail(1)
        emit_finish2(0)
        gen = o_gen(2)
        emit_scores(3, gen)
        for _ in gen:
            pass
        finish1_tail(2)
        emit_finish2(1)
        gen = o_gen(3)
        for _ in range(8):
            next(gen, None)
        emit_finish2(2)
        for _ in gen:
            pass
        finish1_tail(3)
        emit_finish2(3)

        # ---- MLP head over all graphs ----
        h1_ps = ps.tile([128, G], f32, tag="m", name="h1_ps")
        nc.tensor.matmul(h1_ps[:], W["Wf0"][:], feat[:], start=True, stop=True)
        h1 = work.tile([128, G], f32, tag="h1")
        elu(h1[:], h1_ps[:], W["bf0"][:], 128, G, "m1")
        h2_ps = ps.tile([64, G], f32, tag="m", name="h2_ps")
        nc.tensor.matmul(h2_ps[:], W["Wf1"][:], h1[:], start=True, stop=True)
        h2 = work.tile([64, G], f32, tag="h2")
        elu(h2[:], h2_ps[:], W["bf1"][:], 64, G, "m2")
        h3_ps = ps.tile([1, G], f32, tag="m", name="h3_ps")
        nc.tensor.matmul(h3_ps[:], W["Wf2"][:], h2[:], start=True, stop=True)
        if debug:
            nc.sync.dma_start(dbg_d["d_x0"][:], x0[:])
            nc.sync.dma_start(dbg_d["d_qtA"][:], qtA[:])
            nc.sync.dma_start(dbg_d["d_ktA"][:], ktA[:])
            nc.sync.dma_start(dbg_d["d_x1"][:], x1[:])
            nc.sync.dma_start(dbg_d["d_feat"][:], feat[:])
        fout = work.tile([1, G], f32, tag="fout")
        # elu in f32 for the final scaled output
        e = work.tile([1, G], f32, tag="fin_e")
        nc.scalar.activation(e[:], h3_ps[:], AF.Exp, bias=W["bf2"][:])
        nc.vector.tensor_scalar(e[:], e[:], 1.0, 0.0, ALU.subtract, ALU.min)
        r = work.tile([1, G], f32, tag="fin_r")
        nc.scalar.activation(r[:], h3_ps[:], AF.Relu, bias=W["bf2"][:])
        nc.vector.tensor_add(fout[:], e[:], r[:])
        nc.vector.tensor_scalar_mul(out_sb[:], fout[:], float(SCALE))
        nc.sync.dma_start(out_d[:], out_sb[:])

    nc.compile()
    return nc


# ======================= host-side marshaling =======================

def _bf16(x):
    import ml_dtypes
    return np.asarray(x, dtype=np.float32).astype(ml_dtypes.bfloat16)


def _prep_weights(inputs):
    f32 = np.float32
    w = {}

    def headcols(Wm, bv, stride, scale=1.0, heads=range(H), ones_col=False):
        # [din+1, 128] with head h (enumerated j) at column stride*j
        din = Wm.shape[0]
        O = np.zeros((din + 1, P), f32)
        for j, h in enumerate(heads):
            O[0:din, stride * j:stride * j + Wm.shape[2]] = Wm[:, h, :] * scale
            O[din, stride * j:stride * j + Wm.shape[2]] = bv[h, :] * scale
            if ones_col:
                O[din, stride * j + DO] = 1.0
        return O

    Wq0 = np.asarray(inputs["Wq0"], f32); bq0 = np.asarray(inputs["bq0"], f32)
    Wk0 = np.asarray(inputs["Wk0"], f32); bk0 = np.asarray(inputs["bk0"], f32)
    Wv0 = np.asarray(inputs["Wv0"], f32); bv0 = np.asarray(inputs["bv0"], f32)
    s = 1.0 / np.sqrt(DH)
    w["WqA"] = headcols(Wq0, bq0, 32, heads=range(0, 4))
    w["WqB"] = headcols(Wq0, bq0, 32, heads=range(4, 8))
    w["WkA"] = headcols(Wk0, bk0, 32, scale=s, heads=range(0, 4))
    w["WkB"] = headcols(Wk0, bk0, 32, scale=s, heads=range(4, 8))
    w["WvAB"] = np.concatenate(
        [headcols(Wv0, bv0, 32, heads=range(0, 4), ones_col=True),
         headcols(Wv0, bv0, 32, heads=range(4, 8), ones_col=True)], axis=1)

    Wl0 = np.asarray(inputs["Wl0"], f32)  # [H*DO, DLIN]
    for half, nm in ((0, "WlA"), (1, "WlB")):
        O = np.zeros((P, DLIN), f32)
        for j in range(4):
            h = 4 * half + j
            O[32 * j:32 * j + DO, :] = Wl0[DO * h:DO * (h + 1), :]
        w[nm] = O
    for half, nm in ((0, "SelA"), (1, "SelB")):
        O = np.zeros((P, H), f32)
        for j in range(4):
            O[32 * j + DO, 4 * half + j] = 1.0
        w[nm] = O
    for half, nm in ((0, "EA"), (1, "EB")):
        O = np.zeros((H, P), f32)
        for j in range(4):
            O[4 * half + j, 32 * j:32 * j + DO] = 1.0
        w[nm] = O

    Wq1 = np.asarray(inputs["Wq1"], f32); bq1 = np.asarray(inputs["bq1"], f32)
    Wk1 = np.asarray(inputs["Wk1"], f32); bk1 = np.asarray(inputs["bk1"], f32)
    Wv1 = np.asarray(inputs["Wv1"], f32); bv1 = np.asarray(inputs["bv1"], f32)
    w["Wq2"] = headcols(Wq1, bq1, DH)
    w["Wk2"] = headcols(Wk1, bk1, DH, scale=s)
    Wv2 = headcols(Wv1, bv1, DH)
    I64a = np.concatenate([np.eye(DLIN, dtype=f32),
                           np.zeros((1, DLIN), f32)], axis=0)
    w["Wv2I"] = np.concatenate([Wv2, I64a], axis=1)
    Wl1 = np.asarray(inputs["Wl1"], f32)
    O = np.zeros((P, DLIN), f32)
    for h in range(H):
        O[DH * h:DH * h + DO, :] = Wl1[DO * h:DO * (h + 1), :]
    w["Wl2"] = O
    E2 = np.zeros((H, P), f32)
    for h in range(H):
        E2[h, DH * h:DH * h + DO] = 1.0
    w["E2"] = E2
    bm = np.zeros((P, H), f32)
    for h in range(H):
        bm[DH * h:DH * h + DH, h] = 1.0
    w["bm16"] = bm
    w["ones"] = np.ones((P, 1), f32)
    w["Wi"] = np.concatenate([np.asarray(inputs["W_init"], f32),
                              np.zeros((1, DINIT), f32)], axis=0)
    w["Wi"][DIN, :] = 0.0  # bias folded separately via bi (ACT bias)
    w["Wf0"] = np.asarray(inputs["Wf0"], f32)
    w["Wf1"] = np.asarray(inputs["Wf1"], f32)
    w["Wf2"] = np.asarray(inputs["Wf2"], f32)

    F32W = ("Wq2", "Wl2", "E2", "Wf0", "Wf1", "Wf2")
    out = {k: (np.asarray(v, np.float32) if k in F32W else _bf16(v))
           for k, v in w.items()}
    out["bi"] = np.asarray(inputs["b_init"], np.float32).reshape(DINIT, 1)
    out["bl0"] = np.asarray(inputs["bl0"], np.float32).reshape(DLIN, 1)
    out["bl1"] = np.asarray(inputs["bl1"], np.float32).reshape(DLIN, 1)
    out["bf0"] = np.asarray(inputs["bf0"], np.float32).reshape(128, 1)
    out["bf1"] = np.asarray(inputs["bf1"], np.float32).reshape(64, 1)
    out["bf2"] = np.asarray(inputs["bf2"], np.float32).reshape(1, 1)
    return out


def _prep_core_inputs(inputs, core):
    f32 = np.float32
    sl = slice(core * G, (core + 1) * G)
    nfi = np.asarray(inputs["node_features"], f32)[sl]     # [G, N, DIN]
    adj = np.asarray(inputs["adj"], f32)[sl]               # [G, N, N]
    masks = np.asarray(inputs["masks"], f32)[sl]           # [G, N]
    qidx = np.asarray(inputs["query_idxs"])[sl]            # [G]

    # nf: [DIN+1, G*N]; row DIN = 1 (augmentation for bias rows of W*)
    nf = np.concatenate([np.transpose(nfi, (0, 2, 1)),
                         np.ones((G, 1, N), f32)], axis=1)   # [G, 17, N]
    nf = np.transpose(nf, (1, 0, 2)).reshape(DIN + 1, G * N)

    # adjb[g, mc, p, n] = keymask(m = mc*128+p source, n dest)
    adjT = ((np.transpose(adj, (0, 2, 1)) > 0) & (masks[:, :, None] > 0))
    adjb = adjT.astype(f32).reshape(G, MC, P, N)

    adjq = np.stack([(adj[g, qidx[g]] > 0) & (masks[g] > 0) for g in range(G)])
    adjq = adjq.astype(f32).reshape(G, MC, P).transpose(0, 2, 1)  # [G, P, MC]
    onehot = np.zeros((G, N), f32)
    onehot[np.arange(G), qidx] = 1.0
    onehot = onehot.reshape(G, MC, P).transpose(0, 2, 1)

    return {
        "nf": _bf16(nf),
        "adjb": _bf16(adjb),
        "adjq": _bf16(adjq),
        "oh": np.asarray(onehot, np.float32),
    }


def kernel(**inputs) -> np.ndarray:
    from concourse.bass_utils import run_bass_kernel_spmd

    nc = _build_nc()
    w = _prep_weights(inputs)
    in_maps = []
    for core in range(NCORES):
        m = _prep_core_inputs(inputs, core)
        m.update(w)
        in_maps.append(m)
    res = run_bass_kernel_spmd(nc, in_maps, list(range(NCORES)))
    out = np.concatenate([res.results[i]["out"][0] for i in range(NCORES)])
    return out.astype(np.float32).reshape(B, 1)


# revision 5
# speedup vs baseline: 1.0183x; 1.0095x over previous
"""Trainium2 Bass kernel for DenseGatPerfPlayerModel — v2 (bf16 + PE tiling).

Design (8 NeuronCores, 4 graphs/core, data-parallel over B):
  - ALL fat matmuls in bf16 (fp32 moving operand costs 4 cyc/row; bf16 1).
  - Heads split in two halves (A: 0-3, B: 4-7), head j of a half at partition
    base 32j.  Score matmuls are K=16 row-tiles (tile_position=(32j,0)) so up
    to 4 heads stream concurrently through distinct 32-row strips of the PE.
  - o matmuls are M=32 col-tiles (tile_position=(0,32j)): 4 heads of a half
    accumulate into ONE psum bank at 32-partition stride.  Softmax denominator
    comes for free from a ones-column folded into the Wv weights (bias-row
    trick): o row 32j+16 = sum_m p[m,n].
  - Projections (x0/q/k/v/k2/v2/...) batched across all 4 graphs (free dim
    2048) with weights as lhsT -> one LDWEIGHTS amortized over 4 matmuls.
  - exp on ScalarE (only engine with exp), psum->sbuf bf16; adjacency mask
    multiply split between VectorE and GpSimd (bf16 2x mode on DVE).
  - Layer 2 evaluated only at the query node via block-diagonal tricks:
    s2[m,h] in ONE matmul per (g,chunk) (lhsT=k2t chunk [128(he),128(m)],
    rhs=block-diag q2), o2[(h,e),h'] via 16-stride v2, diagonal extracted with
    a mask+reduce.  den2 = ones^T pm2 matmul.
  - PSUM budget (8 banks): scores [128,2,512]x2bufs = 4, o [128,512]x2 = 2,
    misc [128,512]x2 = 2.
"""

import numpy as np

B, N = 32, 512
G = 4
NCORES = 8
H, DH, DO, DLIN = 8, 16, 16, 64
DIN, DINIT = 16, 64
SCALE = 1999853.335557038
P = 128
MC = N // P


def _build_nc(debug=False):
    from contextlib import ExitStack

    import concourse.mybir as mybir
    import concourse.tile as tile
    from concourse import bacc

    f32 = mybir.dt.float32
    bf = mybir.dt.bfloat16
    AF = mybir.ActivationFunctionType
    ALU = mybir.AluOpType

    nc = bacc.Bacc()

    # ---- DRAM parameters (per-core shard) ----
    adjb_d = nc.declare_dram_parameter("adjb", [G, MC, P, N], bf, isOutput=False)
    nf_d = nc.declare_dram_parameter("nf", [DIN + 1, G * N], bf, isOutput=False)
    adjq_d = nc.declare_dram_parameter("adjq", [G, P, MC], bf, isOutput=False)
    oh_d = nc.declare_dram_parameter("oh", [G, P, MC], f32, isOutput=False)

    wspec_bf = {
        "Wi": [DIN + 1, DINIT],
        "WqA": [DINIT + 1, P], "WqB": [DINIT + 1, P],
        "WkA": [DINIT + 1, P], "WkB": [DINIT + 1, P],
        "WvAB": [DINIT + 1, 256],
        "WlA": [P, DLIN], "WlB": [P, DLIN],
        "SelA": [P, H], "SelB": [P, H],
        "EA": [H, P], "EB": [H, P],
        "Wk2": [DLIN + 1, P], "Wv2I": [DLIN + 1, P + DLIN],
        "bm16": [P, H], "ones": [P, 1],
    }
    wspec_f32 = {
        "Wq2": [DLIN + 1, P], "Wl2": [P, DLIN], "E2": [H, P],
        "Wf0": [2 * DLIN, 128], "Wf1": [128, 64], "Wf2": [64, 1],
        "bl0": [DLIN, 1], "bl1": [DLIN, 1],
        "bi": [DINIT, 1],
        "bf0": [128, 1], "bf1": [64, 1], "bf2": [1, 1],
    }
    w_d = {k: nc.declare_dram_parameter(k, s, bf, isOutput=False)
           for k, s in wspec_bf.items()}
    w_d.update({k: nc.declare_dram_parameter(k, s, f32, isOutput=False)
                for k, s in wspec_f32.items()})
    out_d = nc.declare_dram_parameter("out", [1, G], f32, isOutput=True)
    if debug:
        dbg_d = {
            "d_x0": nc.declare_dram_parameter("d_x0", [DINIT + 1, G * N], bf, isOutput=True),
            "d_qtA": nc.declare_dram_parameter("d_qtA", [P, G * N], bf, isOutput=True),
            "d_ktA": nc.declare_dram_parameter("d_ktA", [P, G * N], bf, isOutput=True),
            "d_osbA": nc.declare_dram_parameter("d_osbA", [P, N], bf, isOutput=True),
            "d_x1": nc.declare_dram_parameter("d_x1", [DLIN + 1, G * N], bf, isOutput=True),
            "d_feat": nc.declare_dram_parameter("d_feat", [2 * DLIN, G], f32, isOutput=True),
        }

    with tile.TileContext(nc) as tc, ExitStack() as ctx:
        wpool = ctx.enter_context(tc.tile_pool(name="w", bufs=1))
        data = ctx.enter_context(tc.tile_pool(name="data", bufs=1))
        work = ctx.enter_context(tc.tile_pool(name="work", bufs=3))
        pmpool = ctx.enter_context(tc.tile_pool(name="pm", bufs=1))
        osb = ctx.enter_context(tc.tile_pool(name="osb", bufs=2))
        ps = ctx.enter_context(tc.tile_pool(name="ps", bufs=2, space="PSUM"))

        # ---- weights + inputs to SBUF (critical-path first, 3 queues) ----
        dma_engines = [nc.sync]
        dma_ctr = [0]

        def dma(dst, src):
            eng = dma_engines[dma_ctr[0] % len(dma_engines)]
            dma_ctr[0] += 1
            eng.dma_start(dst, src)

        W = {}
        for k in list(wspec_bf) + list(wspec_f32):
            dt = bf if k in wspec_bf else f32
            shape = wspec_bf.get(k) or wspec_f32[k]
            W[k] = wpool.tile(shape, dt, tag=f"w_{k}", name=f"w_{k}")
        nf = data.tile([DIN + 1, G * N], bf, tag="nf")
        adjb = data.tile([P, G * MC, N], bf, tag="adjb")
        adjq_sb = wpool.tile([P, G, MC], bf, tag="adjq")
        oh_sb = wpool.tile([P, G, MC], f32, tag="oh")
        # phase-A critical path first
        dma(nf[:], nf_d[:])
        for k in ("Wi", "bi", "WqA", "WkA", "WqB", "WkB", "WvAB"):
            dma(W[k][:], w_d[k][:])
        for g in range(G):
            for mc in range(MC):
                dma(adjb[:, g * MC + mc, :], adjb_d[g, mc])
        for k in list(wspec_bf) + list(wspec_f32):
            if k in ("Wi", "bi", "WqA", "WkA", "WqB", "WkB", "WvAB"):
                continue
            dma(W[k][:], w_d[k][:])
        for g in range(G):
            dma(adjq_sb[:, g, :], adjq_d[g])
            dma(oh_sb[:, g, :], oh_d[g])

        # persistent SBUF state
        x0 = data.tile([DINIT + 1, G * N], bf, tag="x0")
        x1 = data.tile([DLIN + 1, G * N], bf, tag="x1")
        qtA = data.tile([P, G * N], bf, tag="qtA")
        qtB = data.tile([P, G * N], bf, tag="qtB")
        ktA = data.tile([P, G * N], bf, tag="ktA")
        ktB = data.tile([P, G * N], bf, tag="ktB")
        vsb = data.tile([P, G * MC, 256], bf, tag="vsb")  # A cols 0:128, B 128:256
        k2t = data.tile([P, G * N], bf, tag="k2t")
        v2sb = data.tile([P, G * MC, P], bf, tag="v2sb")
        x1nd = data.tile([P, G * MC, DLIN], f32, tag="x1nd")
        x1qa = data.tile([DLIN + 1, G], f32, tag="x1qa")
        q2bd = data.tile([P, G, H], bf, tag="q2bd")
        feat = data.tile([2 * DLIN, G], f32, tag="feat")
        out_sb = data.tile([1, G], f32, tag="out_sb")
        zeros = data.tile([P, 1], bf, tag="zeros")
        nc.vector.memset(zeros[:], 0.0)
        nc.vector.memset(x0[DINIT:DINIT + 1, :], 1.0)
        nc.vector.memset(x1[DLIN:DLIN + 1, :], 1.0)
        nc.vector.memset(x1qa[DLIN:DLIN + 1, :], 1.0)
        warm = data.tile([1, 1], f32, tag="warm")
        nc.scalar.activation(warm[:], zeros[0:1, 0:1], AF.Exp)

        mask_ctr = [0]

        def mask_engine():
            # split mask multiplies ~3:2 between DVE and GpSimd
            mask_ctr[0] += 1
            return nc.vector if mask_ctr[0] % 3 < 2 else nc.gpsimd

    # --- helpers -------------------------------------------------------
        def elu(dst, src_ps, bias_ap, p, f, tg):
            # dst(bf16) = elu(src_ps + bias); src_ps is PSUM [p, f]
            e = work.tile([p, f], f32, tag=f"elu_e_{tg}", name=f"elu_e_{tg}", bufs=1)
            nc.scalar.activation(e[:], src_ps, AF.Exp, bias=bias_ap)
            e2 = work.tile([p, f], f32, tag=f"elu_e2_{tg}", name=f"elu_e2_{tg}", bufs=1)
            nc.vector.tensor_scalar(e2[:], e[:], 1.0, 0.0, ALU.subtract, ALU.min)
            r = work.tile([p, f], f32, tag=f"elu_r_{tg}", name=f"elu_r_{tg}", bufs=1)
            nc.scalar.activation(r[:], src_ps, AF.Relu, bias=bias_ap)
            nc.vector.tensor_add(dst, e2[:], r[:])

        # ---- phase A: x0 = elu(nf @ Wi + bi), then q/k/v projections ----
        for t in range(2):
            sp = ps.tile([P, 2, N], f32, tag="s", name="x0ps")
            for r in range(2):
                c = (2 * t + r) * N
                nc.tensor.matmul(sp[0:DINIT, r, :], W["Wi"][:], nf[:, c:c + N],
                                 start=True, stop=True)
            elu(x0[0:DINIT, 2 * t * N:(2 * t + 2) * N],
                sp[0:DINIT, :, :].rearrange("p a n -> p (a n)"),
                W["bi"][:], DINIT, 2 * N, "x0")

        for t in range(2):
            for wn, dst in (("WqA", qtA), ("WkA", ktA), ("WqB", qtB), ("WkB", ktB)):
                sp = ps.tile([P, 2, N], f32, tag="s", name=f"p_{wn}{t}")
                for r in range(2):
                    c = (2 * t + r) * N
                    nc.tensor.matmul(sp[:, r, :], W[wn][:], x0[:, c:c + N],
                                     start=True, stop=True)
                nc.vector.tensor_copy(
                    dst[:, 2 * t * N:(2 * t + 2) * N],
                    sp.rearrange("p a n -> p (a n)"))

        def emit_v():
            for g in range(G):
                for mc in range(MC):
                    vp = ps.tile([P, 256], f32, tag="m", name="vp")
                    c = g * N + mc * P
                    nc.tensor.matmul(vp[:], x0[:, c:c + P], W["WvAB"][:],
                                     start=True, stop=True)
                    nc.vector.tensor_copy(vsb[:, g * MC + mc, :], vp[:])

        # ---- phase B+C: software-pipelined over graphs ----
        pm_refs = {}
        o_ps_refs = {}

        def emit_scores(g, filler=None):
            gc = g * N
            pm_ref = {}
            for half, (qt, kt) in enumerate(((qtA, ktA), (qtB, ktB))):
                for mc in range(MC):
                    for pair in range(2):
                        sp = ps.tile([P, 2, N], f32, tag="s", name="sp")
                        for r in range(2):
                            j = 2 * pair + r
                            nc.tensor.matmul(
                                sp[:, r, :],
                                kt[32 * j:32 * j + DH, gc + mc * P:gc + (mc + 1) * P],
                                qt[32 * j:32 * j + DH, gc:gc + N],
                                start=True, stop=True, tile_position=(32 * j, 0))
                        ex = work.tile([P, 2, N], bf, tag="ex")
                        nc.scalar.activation(ex[:], sp[:], AF.Exp)
                        pmt = pmpool.tile([P, 2, N], bf,
                                          tag=f"pm{g % 2}_{half}_{mc}_{pair}",
                                          name=f"pm{g}_{half}_{mc}_{pair}")
                        mask_engine().tensor_tensor(
                            pmt[:], ex[:],
                            adjb[:, g * MC + mc, None, :].to_broadcast((P, 2, N)),
                            ALU.mult)
                        pm_ref[(half, mc, pair)] = pmt
                        if filler is not None:
                            next(filler, None)
            pm_refs[g] = pm_ref

        def o_gen(g):
            pm_ref = pm_refs.pop(g)
            o_psA = ps.tile([P, N], f32, tag="o", name="opsA")
            o_psB = ps.tile([P, N], f32, tag="o", name="opsB")
            o_ps_refs[g] = (o_psA, o_psB)
            for ja in range(4):
                jb = (ja + 1) % 4
                for mc in range(MC):
                    for half, o_ps, j in ((0, o_psA, ja), (1, o_psB, jb)):
                        nc.tensor.matmul(
                            o_ps[32 * j:32 * j + 32, :],
                            vsb[:, g * MC + mc, P * half + 32 * j:P * half + 32 * j + 32],
                            pm_ref[(half, mc, j // 2)][:, j % 2, :],
                            start=(mc == 0), stop=(mc == MC - 1),
                            tile_position=(0, 32 * j))
                    yield

        def finish1_tail(g):
            gc = g * N
            o_psA, o_psB = o_ps_refs.pop(g)
            o_sb_ref = {}
            for half, o_ps in ((0, o_psA), (1, o_psB)):
                o_sbt = osb.tile([P, N], bf, tag=f"osb{half}", name=f"osb{half}")
                nc.vector.tensor_copy(o_sbt[:], o_ps[:])
                o_sb_ref[half] = o_sbt

            # normalize + Wl + elu -> x1
            den_ps = ps.tile([H, N], f32, tag="m", name="den_ps")
            nc.tensor.matmul(den_ps[:], W["SelA"][:], o_sb_ref[0][:],
                             start=True, stop=False)
            nc.tensor.matmul(den_ps[:], W["SelB"][:], o_sb_ref[1][:],
                             start=False, stop=True)
            recf = work.tile([H, N], f32, tag="recf")
            nc.vector.reciprocal_approx_fast(recf[:], den_ps[:])
            den_sb = work.tile([H, N], bf, tag="den_sb")
            nc.vector.tensor_copy(den_sb[:], recf[:])
            scr_ref = {}
            for half in range(2):
                db_ps = ps.tile([P, N], f32, tag="m", name="db_ps")
                nc.tensor.matmul(db_ps[:], W["EA" if half == 0 else "EB"][:],
                                 den_sb[:], start=True, stop=True)
                scr = work.tile([P, N], bf, tag=f"scr{half}", name=f"scr{half}",
                                bufs=2)
                nc.vector.scalar_tensor_tensor(
                    scr[:], o_sb_ref[half][:], 1.0, db_ps[:], ALU.mult, ALU.mult)
                scr_ref[half] = scr
            x1_ps = ps.tile([DLIN, N], f32, tag="m", name="x1_ps")
            nc.tensor.matmul(x1_ps[:], W["WlA"][:], scr_ref[0][:],
                             start=True, stop=False)
            nc.tensor.matmul(x1_ps[:], W["WlB"][:], scr_ref[1][:],
                             start=False, stop=True)
            elu(x1[0:DLIN, gc:gc + N], x1_ps[:], W["bl0"][:], DLIN, N, "x1")

        def emit_finish2(g):
            gc = g * N
            # ---- layer 2 for this graph ----
            # k2t / v2 / x1nd chunks
            kp = ps.tile([P, N], f32, tag="m", name="k2p")
            nc.tensor.matmul(kp[:], W["Wk2"][:], x1[:, gc:gc + N],
                             start=True, stop=True)
            nc.vector.tensor_copy(k2t[:, gc:gc + N], kp[:])
            for mc in range(MC):
                c = gc + mc * P
                vp2 = ps.tile([P, P + DLIN], f32, tag="m", name="v2p")
                nc.tensor.matmul(vp2[:], x1[:, c:c + P], W["Wv2I"][:],
                                 start=True, stop=True)
                nc.vector.tensor_copy(v2sb[:, g * MC + mc, :], vp2[:, 0:P])
                nc.vector.tensor_copy(x1nd[:, g * MC + mc, :], vp2[:, P:P + DLIN])
            # x1 at query node
            xq_ps = ps.tile([DLIN, MC], f32, tag="m", name="xq_ps")
            for mc in range(MC):
                nc.tensor.matmul(xq_ps[:, mc:mc + 1], x1nd[:, g * MC + mc, :],
                                 oh_sb[:, g, mc:mc + 1], start=True, stop=True)
            xq = work.tile([DLIN, 1], f32, tag="xq")
            nc.vector.tensor_reduce(xq[:], xq_ps[:], mybir.AxisListType.X,
                                    ALU.add)
            nc.vector.tensor_copy(feat[0:DLIN, g:g + 1], xq[:])
            nc.gpsimd.tensor_copy(x1qa[0:DLIN, g:g + 1], xq[:])
            # q2 block-diag
            q2_ps = ps.tile([P, 1], f32, tag="m", name="q2_ps")
            nc.tensor.matmul(q2_ps[:], W["Wq2"][:], x1qa[:, g:g + 1],
                             start=True, stop=True)
            nc.vector.scalar_tensor_tensor(
                q2bd[:, g, :], q2_ps[:, 0:1].to_broadcast((P, H)), 1.0,
                W["bm16"][:], ALU.mult, ALU.mult)
            # attention at query node: s2/o2/den2 (mc in free dim, no groups)
            o2_ps = ps.tile([P, MC, H], f32, tag="m", name="o2_ps")
            d2_ps = ps.tile([H, MC], f32, tag="m", name="d2_ps")
            for mc in range(MC):
                c = gc + mc * P
                s2_ps = ps.tile([P, 2, N], f32, tag="s", name="s2_ps")
                nc.tensor.matmul(s2_ps[:, 0, 0:H], k2t[:, c:c + P],
                                 q2bd[:, g, :], start=True, stop=True)
                ex2 = work.tile([P, H], bf, tag="ex2")
                nc.scalar.activation(ex2[:], s2_ps[:, 0, 0:H], AF.Exp)
                pm2 = work.tile([P, H], bf, tag="pm2")
                nc.vector.tensor_tensor(
                    pm2[:], ex2[:],
                    adjq_sb[:, g, mc:mc + 1].to_broadcast((P, H)), ALU.mult)
                nc.tensor.matmul(o2_ps[:, mc, :], v2sb[:, g * MC + mc, :],
                                 pm2[:], start=True, stop=True)
                nc.tensor.matmul(d2_ps[:, mc:mc + 1], pm2[:], W["ones"][:],
                                 start=True, stop=True)
            # extract diag blocks + denominators
            o2m = work.tile([P, MC, H], f32, tag="o2m")
            nc.vector.tensor_tensor(
                o2m[:], o2_ps[:],
                W["bm16"][:, None, :].to_broadcast((P, MC, H)), ALU.mult)
            o2v = work.tile([P, 1], f32, tag="o2v")
            nc.vector.tensor_reduce(o2v[:], o2m[:], mybir.AxisListType.XY,
                                    ALU.add)
            den2 = work.tile([H, 1], f32, tag="den2")
            nc.vector.tensor_reduce(den2[:], d2_ps[:], mybir.AxisListType.X,
                                    ALU.add)
            rec2 = work.tile([H, 1], f32, tag="rec2")
            nc.vector.reciprocal(rec2[:], den2[:])
            scb_ps = ps.tile([P, 1], f32, tag="m", name="scb_ps")
            nc.tensor.matmul(scb_ps[:], W["E2"][:], rec2[:],
                             start=True, stop=True)
            scr2 = work.tile([P, 1], f32, tag="scr2")
            nc.vector.scalar_tensor_tensor(scr2[:], scb_ps[:], 1.0, o2v[:],
                                           ALU.mult, ALU.mult)
            x2_ps = ps.tile([DLIN, 1], f32, tag="m", name="x2_ps")
            nc.tensor.matmul(x2_ps[:], W["Wl2"][:], scr2[:],
                             start=True, stop=True)
            elu(feat[DLIN:2 * DLIN, g:g + 1], x2_ps[:], W["bl1"][:],
                DLIN, 1, "x2")

        emit_scores(0)
        emit_v()
        gen = o_gen(0)
        emit_scores(1, gen)
        for _ in gen:
            pass
        finish1_tail(0)
        gen = o_gen(1)
        emit_scores(2, gen)
        for _ in gen:
            pass
        finish1_tail(1)
        emit_finish2(0)
        gen = o_gen(2)
        emit_scores(3, gen)
        for _ in gen:
            pass
        finish1_tail(2)
        emit_finish2(1)
        gen = o_gen(3)
        for _ in range(8):
            next(gen, None)
        emit_finish2(2)
        for _ in gen:
            pass
        finish1_tail(3)
        emit_finish2(3)

        # ---- MLP head over all graphs ----
        h1_ps = ps.tile([128, G], f32, tag="m", name="h1_ps")
        nc.tensor.matmul(h1_ps[:], W["Wf0"][:], feat[:], start=True, stop=True)
        h1 = work.tile([128, G], f32, tag="h1")
        elu(h1[:], h1_ps[:], W["bf0"][:], 128, G, "m1")
        h2_ps = ps.tile([64, G], f32, tag="m", name="h2_ps")
        nc.tensor.matmul(h2_ps[:], W["Wf1"][:], h1[:], start=True, stop=True)
        h2 = work.tile([64, G], f32, tag="h2")
        elu(h2[:], h2_ps[:], W["bf1"][:], 64, G, "m2")
        h3_ps = ps.tile([1, G], f32, tag="m", name="h3_ps")
        nc.tensor.matmul(h3_ps[:], W["Wf2"][:], h2[:], start=True, stop=True)
        if debug:
            nc.sync.dma_start(dbg_d["d_x0"][:], x0[:])
            nc.sync.dma_start(dbg_d["d_qtA"][:], qtA[:])
            nc.sync.dma_start(dbg_d["d_ktA"][:], ktA[:])
            nc.sync.dma_start(dbg_d["d_x1"][:], x1[:])
            nc.sync.dma_start(dbg_d["d_feat"][:], feat[:])
        fout = work.tile([1, G], f32, tag="fout")
        # elu in f32 for the final scaled output
        e = work.tile([1, G], f32, tag="fin_e")
        nc.scalar.activation(e[:], h3_ps[:], AF.Exp, bias=W["bf2"][:])
        nc.vector.tensor_scalar(e[:], e[:], 1.0, 0.0, ALU.subtract, ALU.min)
        r = work.tile([1, G], f32, tag="fin_r")
        nc.scalar.activation(r[:], h3_ps[:], AF.Relu, bias=W["bf2"][:])
        nc.vector.tensor_add(fout[:], e[:], r[:])
        nc.vector.tensor_scalar_mul(out_sb[:], fout[:], float(SCALE))
        nc.sync.dma_start(out_d[:], out_sb[:])

    nc.compile()
    return nc


# ======================= host-side marshaling =======================

def _bf16(x):
    import ml_dtypes
    return np.asarray(x, dtype=np.float32).astype(ml_dtypes.bfloat16)


def _prep_weights(inputs):
    f32 = np.float32
    w = {}

    def headcols(Wm, bv, stride, scale=1.0, heads=range(H), ones_col=False):
        # [din+1, 128] with head h (enumerated j) at column stride*j
        din = Wm.shape[0]
        O = np.zeros((din + 1, P), f32)
        for j, h in enumerate(heads):
            O[0:din, stride * j:stride * j + Wm.shape[2]] = Wm[:, h, :] * scale
            O[din, stride * j:stride * j + Wm.shape[2]] = bv[h, :] * scale
            if ones_col:
                O[din, stride * j + DO] = 1.0
        return O

    Wq0 = np.asarray(inputs["Wq0"], f32); bq0 = np.asarray(inputs["bq0"], f32)
    Wk0 = np.asarray(inputs["Wk0"], f32); bk0 = np.asarray(inputs["bk0"], f32)
    Wv0 = np.asarray(inputs["Wv0"], f32); bv0 = np.asarray(inputs["bv0"], f32)
    s = 1.0 / np.sqrt(DH)
    w["WqA"] = headcols(Wq0, bq0, 32, heads=range(0, 4))
    w["WqB"] = headcols(Wq0, bq0, 32, heads=range(4, 8))
    w["WkA"] = headcols(Wk0, bk0, 32, scale=s, heads=range(0, 4))
    w["WkB"] = headcols(Wk0, bk0, 32, scale=s, heads=range(4, 8))
    w["WvAB"] = np.concatenate(
        [headcols(Wv0, bv0, 32, heads=range(0, 4), ones_col=True),
         headcols(Wv0, bv0, 32, heads=range(4, 8), ones_col=True)], axis=1)

    Wl0 = np.asarray(inputs["Wl0"], f32)  # [H*DO, DLIN]
    for half, nm in ((0, "WlA"), (1, "WlB")):
        O = np.zeros((P, DLIN), f32)
        for j in range(4):
            h = 4 * half + j
            O[32 * j:32 * j + DO, :] = Wl0[DO * h:DO * (h + 1), :]
        w[nm] = O
    for half, nm in ((0, "SelA"), (1, "SelB")):
        O = np.zeros((P, H), f32)
        for j in range(4):
            O[32 * j + DO, 4 * half + j] = 1.0
        w[nm] = O
    for half, nm in ((0, "EA"), (1, "EB")):
        O = np.zeros((H, P), f32)
        for j in range(4):
            O[4 * half + j, 32 * j:32 * j + DO] = 1.0
        w[nm] = O

    Wq1 = np.asarray(inputs["Wq1"], f32); bq1 = np.asarray(inputs["bq1"], f32)
    Wk1 = np.asarray(inputs["Wk1"], f32); bk1 = np.asarray(inputs["bk1"], f32)
    Wv1 = np.asarray(inputs["Wv1"], f32); bv1 = np.asarray(inputs["bv1"], f32)
    w["Wq2"] = headcols(Wq1, bq1, DH)
    w["Wk2"] = headcols(Wk1, bk1, DH, scale=s)
    Wv2 = headcols(Wv1, bv1, DH)
    I64a = np.concatenate([np.eye(DLIN, dtype=f32),
                           np.zeros((1, DLIN), f32)], axis=0)
    w["Wv2I"] = np.concatenate([Wv2, I64a], axis=1)
    Wl1 = np.asarray(inputs["Wl1"], f32)
    O = np.zeros((P, DLIN), f32)
    for h in range(H):
        O[DH * h:DH * h + DO, :] = Wl1[DO * h:DO * (h + 1), :]
    w["Wl2"] = O
    E2 = np.zeros((H, P), f32)
    for h in range(H):
        E2[h, DH * h:DH * h + DO] = 1.0
    w["E2"] = E2
    bm = np.zeros((P, H), f32)
    for h in range(H):
        bm[DH * h:DH * h + DH, h] = 1.0
    w["bm16"] = bm
    w["ones"] = np.ones((P, 1), f32)
    w["Wi"] = np.concatenate([np.asarray(inputs["W_init"], f32),
                              np.zeros((1, DINIT), f32)], axis=0)
    w["Wi"][DIN, :] = 0.0  # bias folded separately via bi (ACT bias)
    w["Wf0"] = np.asarray(inputs["Wf0"], f32)
    w["Wf1"] = np.asarray(inputs["Wf1"], f32)
    w["Wf2"] = np.asarray(inputs["Wf2"], f32)

    F32W = ("Wq2", "Wl2", "E2", "Wf0", "Wf1", "Wf2")
    out = {k: (np.asarray(v, np.float32) if k in F32W else _bf16(v))
           for k, v in w.items()}
    out["bi"] = np.asarray(inputs["b_init"], np.float32).reshape(DINIT, 1)
    out["bl0"] = np.asarray(inputs["bl0"], np.float32).reshape(DLIN, 1)
    out["bl1"] = np.asarray(inputs["bl1"], np.float32).reshape(DLIN, 1)
    out["bf0"] = np.asarray(inputs["bf0"], np.float32).reshape(128, 1)
    out["bf1"] = np.asarray(inputs["bf1"], np.float32).reshape(64, 1)
    out["bf2"] = np.asarray(inputs["bf2"], np.float32).reshape(1, 1)
    return out


def _prep_core_inputs(inputs, core):
    f32 = np.float32
    sl = slice(core * G, (core + 1) * G)
    nfi = np.asarray(inputs["node_features"], f32)[sl]     # [G, N, DIN]
    adj = np.asarray(inputs["adj"], f32)[sl]               # [G, N, N]
    masks = np.asarray(inputs["masks"], f32)[sl]           # [G, N]
    qidx = np.asarray(inputs["query_idxs"])[sl]            # [G]

    # nf: [DIN+1, G*N]; row DIN = 1 (augmentation for bias rows of W*)
    nf = np.concatenate([np.transpose(nfi, (0, 2, 1)),
                         np.ones((G, 1, N), f32)], axis=1)   # [G, 17, N]
    nf = np.transpose(nf, (1, 0, 2)).reshape(DIN + 1, G * N)

    # adjb[g, mc, p, n] = keymask(m = mc*128+p source, n dest)
    adjT = ((np.transpose(adj, (0, 2, 1)) > 0) & (masks[:, :, None] > 0))
    adjb = adjT.astype(f32).reshape(G, MC, P, N)

    adjq = np.stack([(adj[g, qidx[g]] > 0) & (masks[g] > 0) for g in range(G)])
    adjq = adjq.astype(f32).reshape(G, MC, P).transpose(0, 2, 1)  # [G, P, MC]
    onehot = np.zeros((G, N), f32)
    onehot[np.arange(G), qidx] = 1.0
    onehot = onehot.reshape(G, MC, P).transpose(0, 2, 1)

    return {
        "nf": _bf16(nf),
        "adjb": _bf16(adjb),
        "adjq": _bf16(adjq),
        "oh": np.asarray(onehot, np.float32),
    }


def kernel(**inputs) -> np.ndarray:
    from concourse.bass_utils import run_bass_kernel_spmd

    nc = _build_nc()
    w = _prep_weights(inputs)
    in_maps = []
    for core in range(NCORES):
        m = _prep_core_inputs(inputs, core)
        m.update(w)
        in_maps.append(m)
    res = run_bass_kernel_spmd(nc, in_maps, list(range(NCORES)))
    out = np.concatenate([res.results[i]["out"][0] for i in range(NCORES)])
    return out.astype(np.float32).reshape(B, 1)


# revision 6
# speedup vs baseline: 1.0209x; 1.0025x over previous
"""Trainium2 Bass kernel for DenseGatPerfPlayerModel — v2 (bf16 + PE tiling).

Design (8 NeuronCores, 4 graphs/core, data-parallel over B):
  - ALL fat matmuls in bf16 (fp32 moving operand costs 4 cyc/row; bf16 1).
  - Heads split in two halves (A: 0-3, B: 4-7), head j of a half at partition
    base 32j.  Score matmuls are K=16 row-tiles (tile_position=(32j,0)) so up
    to 4 heads stream concurrently through distinct 32-row strips of the PE.
  - o matmuls are M=32 col-tiles (tile_position=(0,32j)): 4 heads of a half
    accumulate into ONE psum bank at 32-partition stride.  Softmax denominator
    comes for free from a ones-column folded into the Wv weights (bias-row
    trick): o row 32j+16 = sum_m p[m,n].
  - Projections (x0/q/k/v/k2/v2/...) batched across all 4 graphs (free dim
    2048) with weights as lhsT -> one LDWEIGHTS amortized over 4 matmuls.
  - exp on ScalarE (only engine with exp), psum->sbuf bf16; adjacency mask
    multiply split between VectorE and GpSimd (bf16 2x mode on DVE).
  - Layer 2 evaluated only at the query node via block-diagonal tricks:
    s2[m,h] in ONE matmul per (g,chunk) (lhsT=k2t chunk [128(he),128(m)],
    rhs=block-diag q2), o2[(h,e),h'] via 16-stride v2, diagonal extracted with
    a mask+reduce.  den2 = ones^T pm2 matmul.
  - PSUM budget (8 banks): scores [128,2,512]x2bufs = 4, o [128,512]x2 = 2,
    misc [128,512]x2 = 2.
"""

import numpy as np

B, N = 32, 512
G = 4
NCORES = 8
H, DH, DO, DLIN = 8, 16, 16, 64
DIN, DINIT = 16, 64
SCALE = 1999853.335557038
P = 128
MC = N // P


def _build_nc(debug=False):
    from contextlib import ExitStack

    import concourse.mybir as mybir
    import concourse.tile as tile
    from concourse import bacc

    f32 = mybir.dt.float32
    bf = mybir.dt.bfloat16
    AF = mybir.ActivationFunctionType
    ALU = mybir.AluOpType

    nc = bacc.Bacc()

    # ---- DRAM parameters (per-core shard) ----
    adjb_d = nc.declare_dram_parameter("adjb", [G, MC, P, N], bf, isOutput=False)
    nf_d = nc.declare_dram_parameter("nf", [DIN + 1, G * N], bf, isOutput=False)
    adjq_d = nc.declare_dram_parameter("adjq", [G, P, MC], bf, isOutput=False)
    oh_d = nc.declare_dram_parameter("oh", [G, P, MC], f32, isOutput=False)

    wspec_bf = {
        "Wi": [DIN + 1, DINIT],
        "WqA": [DINIT + 1, P], "WqB": [DINIT + 1, P],
        "WkA": [DINIT + 1, P], "WkB": [DINIT + 1, P],
        "WvAB": [DINIT + 1, 256],
        "WlA": [P, DLIN], "WlB": [P, DLIN],
        "SelA": [P, H], "SelB": [P, H],
        "EA": [H, P], "EB": [H, P],
        "Wk2": [DLIN + 1, P], "Wv2I": [DLIN + 1, P + DLIN],
        "bm16": [P, H], "ones": [P, 1],
    }
    wspec_f32 = {
        "Wq2": [DLIN + 1, P], "Wl2": [P, DLIN], "E2": [H, P],
        "Wf0": [2 * DLIN, 128], "Wf1": [128, 64], "Wf2": [64, 1],
        "bl0": [DLIN, 1], "bl1": [DLIN, 1],
        "bi": [DINIT, 1],
        "bf0": [128, 1], "bf1": [64, 1], "bf2": [1, 1],
    }
    w_d = {k: nc.declare_dram_parameter(k, s, bf, isOutput=False)
           for k, s in wspec_bf.items()}
    w_d.update({k: nc.declare_dram_parameter(k, s, f32, isOutput=False)
                for k, s in wspec_f32.items()})
    out_d = nc.declare_dram_parameter("out", [1, G], f32, isOutput=True)
    if debug:
        dbg_d = {
            "d_x0": nc.declare_dram_parameter("d_x0", [DINIT + 1, G * N], bf, isOutput=True),
            "d_qtA": nc.declare_dram_parameter("d_qtA", [P, G * N], bf, isOutput=True),
            "d_ktA": nc.declare_dram_parameter("d_ktA", [P, G * N], bf, isOutput=True),
            "d_osbA": nc.declare_dram_parameter("d_osbA", [P, N], bf, isOutput=True),
            "d_x1": nc.declare_dram_parameter("d_x1", [DLIN + 1, G * N], bf, isOutput=True),
            "d_feat": nc.declare_dram_parameter("d_feat", [2 * DLIN, G], f32, isOutput=True),
        }

    with tile.TileContext(nc) as tc, ExitStack() as ctx:
        wpool = ctx.enter_context(tc.tile_pool(name="w", bufs=1))
        data = ctx.enter_context(tc.tile_pool(name="data", bufs=1))
        work = ctx.enter_context(tc.tile_pool(name="work", bufs=3))
        pmpool = ctx.enter_context(tc.tile_pool(name="pm", bufs=1))
        osb = ctx.enter_context(tc.tile_pool(name="osb", bufs=2))
        ps = ctx.enter_context(tc.tile_pool(name="ps", bufs=2, space="PSUM"))

        # ---- weights + inputs to SBUF (critical-path first, 3 queues) ----
        dma_engines = [nc.sync]
        dma_ctr = [0]

        def dma(dst, src):
            eng = dma_engines[dma_ctr[0] % len(dma_engines)]
            dma_ctr[0] += 1
            eng.dma_start(dst, src)

        W = {}
        for k in list(wspec_bf) + list(wspec_f32):
            dt = bf if k in wspec_bf else f32
            shape = wspec_bf.get(k) or wspec_f32[k]
            W[k] = wpool.tile(shape, dt, tag=f"w_{k}", name=f"w_{k}")
        nf = data.tile([DIN + 1, G * N], bf, tag="nf")
        adjb = data.tile([P, G * MC, N], bf, tag="adjb")
        adjq_sb = wpool.tile([P, G, MC], bf, tag="adjq")
        oh_sb = wpool.tile([P, G, MC], f32, tag="oh")
        # phase-A critical path first
        dma(nf[:], nf_d[:])
        for k in ("Wi", "bi", "WqA", "WkA", "WqB", "WkB", "WvAB"):
            dma(W[k][:], w_d[k][:])
        for g in range(G):
            for mc in range(MC):
                dma(adjb[:, g * MC + mc, :], adjb_d[g, mc])
        for k in list(wspec_bf) + list(wspec_f32):
            if k in ("Wi", "bi", "WqA", "WkA", "WqB", "WkB", "WvAB"):
                continue
            dma(W[k][:], w_d[k][:])
        for g in range(G):
            dma(adjq_sb[:, g, :], adjq_d[g])
            dma(oh_sb[:, g, :], oh_d[g])

        # persistent SBUF state
        x0 = data.tile([DINIT + 1, G * N], bf, tag="x0")
        x1 = data.tile([DLIN + 1, G * N], bf, tag="x1")
        qtA = data.tile([P, G * N], bf, tag="qtA")
        qtB = data.tile([P, G * N], bf, tag="qtB")
        ktA = data.tile([P, G * N], bf, tag="ktA")
        ktB = data.tile([P, G * N], bf, tag="ktB")
        vsb = data.tile([P, G * MC, 256], bf, tag="vsb")  # A cols 0:128, B 128:256
        k2t = data.tile([P, G * N], bf, tag="k2t")
        v2sb = data.tile([P, G * MC, P], bf, tag="v2sb")
        x1nd = data.tile([P, G * MC, DLIN], f32, tag="x1nd")
        x1qa = data.tile([DLIN + 1, G], f32, tag="x1qa")
        q2bd = data.tile([P, G, H], bf, tag="q2bd")
        feat = data.tile([2 * DLIN, G], f32, tag="feat")
        out_sb = data.tile([1, G], f32, tag="out_sb")
        zeros = data.tile([P, 1], bf, tag="zeros")
        nc.vector.memset(zeros[:], 0.0)
        nc.vector.memset(x0[DINIT:DINIT + 1, :], 1.0)
        nc.vector.memset(x1[DLIN:DLIN + 1, :], 1.0)
        nc.vector.memset(x1qa[DLIN:DLIN + 1, :], 1.0)
        warm = data.tile([1, 1], f32, tag="warm")
        nc.scalar.activation(warm[:], zeros[0:1, 0:1], AF.Exp)

        mask_ctr = [0]

        def mask_engine():
            # split mask multiplies ~3:2 between DVE and GpSimd
            mask_ctr[0] += 1
            return nc.vector if mask_ctr[0] % 3 < 2 else nc.gpsimd

    # --- helpers -------------------------------------------------------
        def elu(dst, src_ps, bias_ap, p, f, tg):
            # dst(bf16) = elu(src_ps + bias); src_ps is PSUM [p, f]
            e = work.tile([p, f], f32, tag=f"elu_e_{tg}", name=f"elu_e_{tg}", bufs=1)
            nc.scalar.activation(e[:], src_ps, AF.Exp, bias=bias_ap)
            e2 = work.tile([p, f], f32, tag=f"elu_e2_{tg}", name=f"elu_e2_{tg}", bufs=1)
            nc.vector.tensor_scalar(e2[:], e[:], 1.0, 0.0, ALU.subtract, ALU.min)
            r = work.tile([p, f], f32, tag=f"elu_r_{tg}", name=f"elu_r_{tg}", bufs=1)
            nc.scalar.activation(r[:], src_ps, AF.Relu, bias=bias_ap)
            nc.vector.tensor_add(dst, e2[:], r[:])

        # ---- phase A: x0 = elu(nf @ Wi + bi), then q/k/v projections ----
        for t in range(2):
            sp = ps.tile([P, 2, N], f32, tag="s", name="x0ps")
            for r in range(2):
                c = (2 * t + r) * N
                nc.tensor.matmul(sp[0:DINIT, r, :], W["Wi"][:], nf[:, c:c + N],
                                 start=True, stop=True)
            elu(x0[0:DINIT, 2 * t * N:(2 * t + 2) * N],
                sp[0:DINIT, :, :].rearrange("p a n -> p (a n)"),
                W["bi"][:], DINIT, 2 * N, "x0")

        for t in range(2):
            for wn, dst in (("WqA", qtA), ("WkA", ktA), ("WqB", qtB), ("WkB", ktB)):
                sp = ps.tile([P, 2, N], f32, tag="s", name=f"p_{wn}{t}")
                for r in range(2):
                    c = (2 * t + r) * N
                    nc.tensor.matmul(sp[:, r, :], W[wn][:], x0[:, c:c + N],
                                     start=True, stop=True)
                nc.vector.tensor_copy(
                    dst[:, 2 * t * N:(2 * t + 2) * N],
                    sp.rearrange("p a n -> p (a n)"))

        def v_gen():
            for g in range(G):
                for mc in range(MC):
                    vp = ps.tile([P, 256], f32, tag="m", name="vp")
                    c = g * N + mc * P
                    nc.tensor.matmul(vp[:], x0[:, c:c + P], W["WvAB"][:],
                                     start=True, stop=True)
                    nc.vector.tensor_copy(vsb[:, g * MC + mc, :], vp[:])
                    yield

        # ---- phase B+C: software-pipelined over graphs ----
        pm_refs = {}
        o_ps_refs = {}

        def emit_scores(g, filler=None):
            gc = g * N
            pm_ref = {}
            for half, (qt, kt) in enumerate(((qtA, ktA), (qtB, ktB))):
                for mc in range(MC):
                    for pair in range(2):
                        sp = ps.tile([P, 2, N], f32, tag="s", name="sp")
                        for r in range(2):
                            j = 2 * pair + r
                            nc.tensor.matmul(
                                sp[:, r, :],
                                kt[32 * j:32 * j + DH, gc + mc * P:gc + (mc + 1) * P],
                                qt[32 * j:32 * j + DH, gc:gc + N],
                                start=True, stop=True, tile_position=(32 * j, 0))
                        ex = work.tile([P, 2, N], bf, tag="ex")
                        nc.scalar.activation(ex[:], sp[:], AF.Exp)
                        pmt = pmpool.tile([P, 2, N], bf,
                                          tag=f"pm{g % 2}_{half}_{mc}_{pair}",
                                          name=f"pm{g}_{half}_{mc}_{pair}")
                        mask_engine().tensor_tensor(
                            pmt[:], ex[:],
                            adjb[:, g * MC + mc, None, :].to_broadcast((P, 2, N)),
                            ALU.mult)
                        pm_ref[(half, mc, pair)] = pmt
                        if filler is not None:
                            next(filler, None)
            pm_refs[g] = pm_ref

        def o_gen(g):
            pm_ref = pm_refs.pop(g)
            o_psA = ps.tile([P, N], f32, tag="o", name="opsA")
            o_psB = ps.tile([P, N], f32, tag="o", name="opsB")
            o_ps_refs[g] = (o_psA, o_psB)
            for ja in range(4):
                jb = (ja + 1) % 4
                for mc in range(MC):
                    for half, o_ps, j in ((0, o_psA, ja), (1, o_psB, jb)):
                        nc.tensor.matmul(
                            o_ps[32 * j:32 * j + 32, :],
                            vsb[:, g * MC + mc, P * half + 32 * j:P * half + 32 * j + 32],
                            pm_ref[(half, mc, j // 2)][:, j % 2, :],
                            start=(mc == 0), stop=(mc == MC - 1),
                            tile_position=(0, 32 * j))
                    yield

        def finish1_tail(g):
            gc = g * N
            o_psA, o_psB = o_ps_refs.pop(g)
            o_sb_ref = {}
            for half, o_ps in ((0, o_psA), (1, o_psB)):
                o_sbt = osb.tile([P, N], bf, tag=f"osb{half}", name=f"osb{half}")
                nc.vector.tensor_copy(o_sbt[:], o_ps[:])
                o_sb_ref[half] = o_sbt

            # normalize + Wl + elu -> x1
            den_ps = ps.tile([H, N], f32, tag="m", name="den_ps")
            nc.tensor.matmul(den_ps[:], W["SelA"][:], o_sb_ref[0][:],
                             start=True, stop=False)
            nc.tensor.matmul(den_ps[:], W["SelB"][:], o_sb_ref[1][:],
                             start=False, stop=True)
            recf = work.tile([H, N], f32, tag="recf")
            nc.vector.reciprocal_approx_fast(recf[:], den_ps[:])
            den_sb = work.tile([H, N], bf, tag="den_sb")
            nc.vector.tensor_copy(den_sb[:], recf[:])
            scr_ref = {}
            for half in range(2):
                db_ps = ps.tile([P, N], f32, tag="m", name="db_ps")
                nc.tensor.matmul(db_ps[:], W["EA" if half == 0 else "EB"][:],
                                 den_sb[:], start=True, stop=True)
                scr = work.tile([P, N], bf, tag=f"scr{half}", name=f"scr{half}",
                                bufs=2)
                nc.vector.scalar_tensor_tensor(
                    scr[:], o_sb_ref[half][:], 1.0, db_ps[:], ALU.mult, ALU.mult)
                scr_ref[half] = scr
            x1_ps = ps.tile([DLIN, N], f32, tag="m", name="x1_ps")
            nc.tensor.matmul(x1_ps[:], W["WlA"][:], scr_ref[0][:],
                             start=True, stop=False)
            nc.tensor.matmul(x1_ps[:], W["WlB"][:], scr_ref[1][:],
                             start=False, stop=True)
            elu(x1[0:DLIN, gc:gc + N], x1_ps[:], W["bl0"][:], DLIN, N, "x1")

        def emit_finish2(g):
            gc = g * N
            # ---- layer 2 for this graph ----
            # k2t / v2 / x1nd chunks
            kp = ps.tile([P, N], f32, tag="m", name="k2p")
            nc.tensor.matmul(kp[:], W["Wk2"][:], x1[:, gc:gc + N],
                             start=True, stop=True)
            nc.vector.tensor_copy(k2t[:, gc:gc + N], kp[:])
            for mc in range(MC):
                c = gc + mc * P
                vp2 = ps.tile([P, P + DLIN], f32, tag="m", name="v2p")
                nc.tensor.matmul(vp2[:], x1[:, c:c + P], W["Wv2I"][:],
                                 start=True, stop=True)
                nc.vector.tensor_copy(v2sb[:, g * MC + mc, :], vp2[:, 0:P])
                nc.vector.tensor_copy(x1nd[:, g * MC + mc, :], vp2[:, P:P + DLIN])
            # x1 at query node
            xq_ps = ps.tile([DLIN, MC], f32, tag="m", name="xq_ps")
            for mc in range(MC):
                nc.tensor.matmul(xq_ps[:, mc:mc + 1], x1nd[:, g * MC + mc, :],
                                 oh_sb[:, g, mc:mc + 1], start=True, stop=True)
            xq = work.tile([DLIN, 1], f32, tag="xq")
            nc.vector.tensor_reduce(xq[:], xq_ps[:], mybir.AxisListType.X,
                                    ALU.add)
            nc.vector.tensor_copy(feat[0:DLIN, g:g + 1], xq[:])
            nc.gpsimd.tensor_copy(x1qa[0:DLIN, g:g + 1], xq[:])
            # q2 block-diag
            q2_ps = ps.tile([P, 1], f32, tag="m", name="q2_ps")
            nc.tensor.matmul(q2_ps[:], W["Wq2"][:], x1qa[:, g:g + 1],
                             start=True, stop=True)
            nc.vector.scalar_tensor_tensor(
                q2bd[:, g, :], q2_ps[:, 0:1].to_broadcast((P, H)), 1.0,
                W["bm16"][:], ALU.mult, ALU.mult)
            # attention at query node: s2/o2/den2 (mc in free dim, no groups)
            o2_ps = ps.tile([P, MC, H], f32, tag="m", name="o2_ps")
            d2_ps = ps.tile([H, MC], f32, tag="m", name="d2_ps")
            for mc in range(MC):
                c = gc + mc * P
                s2_ps = ps.tile([P, 2, N], f32, tag="s", name="s2_ps")
                nc.tensor.matmul(s2_ps[:, 0, 0:H], k2t[:, c:c + P],
                                 q2bd[:, g, :], start=True, stop=True)
                ex2 = work.tile([P, H], bf, tag="ex2")
                nc.scalar.activation(ex2[:], s2_ps[:, 0, 0:H], AF.Exp)
                pm2 = work.tile([P, H], bf, tag="pm2")
                nc.vector.tensor_tensor(
                    pm2[:], ex2[:],
                    adjq_sb[:, g, mc:mc + 1].to_broadcast((P, H)), ALU.mult)
                nc.tensor.matmul(o2_ps[:, mc, :], v2sb[:, g * MC + mc, :],
                                 pm2[:], start=True, stop=True)
                nc.tensor.matmul(d2_ps[:, mc:mc + 1], pm2[:], W["ones"][:],
                                 start=True, stop=True)
            # extract diag blocks + denominators
            o2m = work.tile([P, MC, H], f32, tag="o2m")
            nc.vector.tensor_tensor(
                o2m[:], o2_ps[:],
                W["bm16"][:, None, :].to_broadcast((P, MC, H)), ALU.mult)
            o2v = work.tile([P, 1], f32, tag="o2v")
            nc.vector.tensor_reduce(o2v[:], o2m[:], mybir.AxisListType.XY,
                                    ALU.add)
            den2 = work.tile([H, 1], f32, tag="den2")
            nc.vector.tensor_reduce(den2[:], d2_ps[:], mybir.AxisListType.X,
                                    ALU.add)
            rec2 = work.tile([H, 1], f32, tag="rec2")
            nc.vector.reciprocal(rec2[:], den2[:])
            scb_ps = ps.tile([P, 1], f32, tag="m", name="scb_ps")
            nc.tensor.matmul(scb_ps[:], W["E2"][:], rec2[:],
                             start=True, stop=True)
            scr2 = work.tile([P, 1], f32, tag="scr2")
            nc.vector.scalar_tensor_tensor(scr2[:], scb_ps[:], 1.0, o2v[:],
                                           ALU.mult, ALU.mult)
            x2_ps = ps.tile([DLIN, 1], f32, tag="m", name="x2_ps")
            nc.tensor.matmul(x2_ps[:], W["Wl2"][:], scr2[:],
                             start=True, stop=True)
            elu(feat[DLIN:2 * DLIN, g:g + 1], x2_ps[:], W["bl1"][:],
                DLIN, 1, "x2")

        gen = v_gen()
        emit_scores(0, gen)
        for _ in gen:
            pass
        gen = o_gen(0)
        emit_scores(1, gen)
        for _ in gen:
            pass
        finish1_tail(0)
        gen = o_gen(1)
        emit_scores(2, gen)
        for _ in gen:
            pass
        finish1_tail(1)
        emit_finish2(0)
        gen = o_gen(2)
        emit_scores(3, gen)
        for _ in gen:
            pass
        finish1_tail(2)
        emit_finish2(1)
        gen = o_gen(3)
        for _ in range(8):
            next(gen, None)
        emit_finish2(2)
        for _ in gen:
            pass
        finish1_tail(3)
        emit_finish2(3)

        # ---- MLP head over all graphs ----
        h1_ps = ps.tile([128, G], f32, tag="m", name="h1_ps")
        nc.tensor.matmul(h1_ps[:], W["Wf0"][:], feat[:], start=True, stop=True)
        h1 = work.tile([128, G], f32, tag="h1")
        elu(h1[:], h1_ps[:], W["bf0"][:], 128, G, "m1")
        h2_ps = ps.tile([64, G], f32, tag="m", name="h2_ps")
        nc.tensor.matmul(h2_ps[:], W["Wf1"][:], h1[:], start=True, stop=True)
        h2 = work.tile([64, G], f32, tag="h2")
        elu(h2[:], h2_ps[:], W["bf1"][:], 64, G, "m2")
        h3_ps = ps.tile([1, G], f32, tag="m", name="h3_ps")
        nc.tensor.matmul(h3_ps[:], W["Wf2"][:], h2[:], start=True, stop=True)
        if debug:
            nc.sync.dma_start(dbg_d["d_x0"][:], x0[:])
            nc.sync.dma_start(dbg_d["d_qtA"][:], qtA[:])
            nc.sync.dma_start(dbg_d["d_ktA"][:], ktA[:])
            nc.sync.dma_start(dbg_d["d_x1"][:], x1[:])
            nc.sync.dma_start(dbg_d["d_feat"][:], feat[:])
        fout = work.tile([1, G], f32, tag="fout")
        # elu in f32 for the final scaled output
        e = work.tile([1, G], f32, tag="fin_e")
        nc.scalar.activation(e[:], h3_ps[:], AF.Exp, bias=W["bf2"][:])
        nc.vector.tensor_scalar(e[:], e[:], 1.0, 0.0, ALU.subtract, ALU.min)
        r = work.tile([1, G], f32, tag="fin_r")
        nc.scalar.activation(r[:], h3_ps[:], AF.Relu, bias=W["bf2"][:])
        nc.vector.tensor_add(fout[:], e[:], r[:])
        nc.vector.tensor_scalar_mul(out_sb[:], fout[:], float(SCALE))
        nc.sync.dma_start(out_d[:], out_sb[:])

    nc.compile()
    return nc


# ======================= host-side marshaling =======================

def _bf16(x):
    import ml_dtypes
    return np.asarray(x, dtype=np.float32).astype(ml_dtypes.bfloat16)


def _prep_weights(inputs):
    f32 = np.float32
    w = {}

    def headcols(Wm, bv, stride, scale=1.0, heads=range(H), ones_col=False):
        # [din+1, 128] with head h (enumerated j) at column stride*j
        din = Wm.shape[0]
        O = np.zeros((din + 1, P), f32)
        for j, h in enumerate(heads):
            O[0:din, stride * j:stride * j + Wm.shape[2]] = Wm[:, h, :] * scale
            O[din, stride * j:stride * j + Wm.shape[2]] = bv[h, :] * scale
            if ones_col:
                O[din, stride * j + DO] = 1.0
        return O

    Wq0 = np.asarray(inputs["Wq0"], f32); bq0 = np.asarray(inputs["bq0"], f32)
    Wk0 = np.asarray(inputs["Wk0"], f32); bk0 = np.asarray(inputs["bk0"], f32)
    Wv0 = np.asarray(inputs["Wv0"], f32); bv0 = np.asarray(inputs["bv0"], f32)
    s = 1.0 / np.sqrt(DH)
    w["WqA"] = headcols(Wq0, bq0, 32, heads=range(0, 4))
    w["WqB"] = headcols(Wq0, bq0, 32, heads=range(4, 8))
    w["WkA"] = headcols(Wk0, bk0, 32, scale=s, heads=range(0, 4))
    w["WkB"] = headcols(Wk0, bk0, 32, scale=s, heads=range(4, 8))
    w["WvAB"] = np.concatenate(
        [headcols(Wv0, bv0, 32, heads=range(0, 4), ones_col=True),
         headcols(Wv0, bv0, 32, heads=range(4, 8), ones_col=True)], axis=1)

    Wl0 = np.asarray(inputs["Wl0"], f32)  # [H*DO, DLIN]
    for half, nm in ((0, "WlA"), (1, "WlB")):
        O = np.zeros((P, DLIN), f32)
        for j in range(4):
            h = 4 * half + j
            O[32 * j:32 * j + DO, :] = Wl0[DO * h:DO * (h + 1), :]
        w[nm] = O
    for half, nm in ((0, "SelA"), (1, "SelB")):
        O = np.zeros((P, H), f32)
        for j in range(4):
            O[32 * j + DO, 4 * half + j] = 1.0
        w[nm] = O
    for half, nm in ((0, "EA"), (1, "EB")):
        O = np.zeros((H, P), f32)
        for j in range(4):
            O[4 * half + j, 32 * j:32 * j + DO] = 1.0
        w[nm] = O

    Wq1 = np.asarray(inputs["Wq1"], f32); bq1 = np.asarray(inputs["bq1"], f32)
    Wk1 = np.asarray(inputs["Wk1"], f32); bk1 = np.asarray(inputs["bk1"], f32)
    Wv1 = np.asarray(inputs["Wv1"], f32); bv1 = np.asarray(inputs["bv1"], f32)
    w["Wq2"] = headcols(Wq1, bq1, DH)
    w["Wk2"] = headcols(Wk1, bk1, DH, scale=s)
    Wv2 = headcols(Wv1, bv1, DH)
    I64a = np.concatenate([np.eye(DLIN, dtype=f32),
                           np.zeros((1, DLIN), f32)], axis=0)
    w["Wv2I"] = np.concatenate([Wv2, I64a], axis=1)
    Wl1 = np.asarray(inputs["Wl1"], f32)
    O = np.zeros((P, DLIN), f32)
    for h in range(H):
        O[DH * h:DH * h + DO, :] = Wl1[DO * h:DO * (h + 1), :]
    w["Wl2"] = O
    E2 = np.zeros((H, P), f32)
    for h in range(H):
        E2[h, DH * h:DH * h + DO] = 1.0
    w["E2"] = E2
    bm = np.zeros((P, H), f32)
    for h in range(H):
        bm[DH * h:DH * h + DH, h] = 1.0
    w["bm16"] = bm
    w["ones"] = np.ones((P, 1), f32)
    w["Wi"] = np.concatenate([np.asarray(inputs["W_init"], f32),
                              np.zeros((1, DINIT), f32)], axis=0)
    w["Wi"][DIN, :] = 0.0  # bias folded separately via bi (ACT bias)
    w["Wf0"] = np.asarray(inputs["Wf0"], f32)
    w["Wf1"] = np.asarray(inputs["Wf1"], f32)
    w["Wf2"] = np.asarray(inputs["Wf2"], f32)

    F32W = ("Wq2", "Wl2", "E2", "Wf0", "Wf1", "Wf2")
    out = {k: (np.asarray(v, np.float32) if k in F32W else _bf16(v))
           for k, v in w.items()}
    out["bi"] = np.asarray(inputs["b_init"], np.float32).reshape(DINIT, 1)
    out["bl0"] = np.asarray(inputs["bl0"], np.float32).reshape(DLIN, 1)
    out["bl1"] = np.asarray(inputs["bl1"], np.float32).reshape(DLIN, 1)
    out["bf0"] = np.asarray(inputs["bf0"], np.float32).reshape(128, 1)
    out["bf1"] = np.asarray(inputs["bf1"], np.float32).reshape(64, 1)
    out["bf2"] = np.asarray(inputs["bf2"], np.float32).reshape(1, 1)
    return out


def _prep_core_inputs(inputs, core):
    f32 = np.float32
    sl = slice(core * G, (core + 1) * G)
    nfi = np.asarray(inputs["node_features"], f32)[sl]     # [G, N, DIN]
    adj = np.asarray(inputs["adj"], f32)[sl]               # [G, N, N]
    masks = np.asarray(inputs["masks"], f32)[sl]           # [G, N]
    qidx = np.asarray(inputs["query_idxs"])[sl]            # [G]

    # nf: [DIN+1, G*N]; row DIN = 1 (augmentation for bias rows of W*)
    nf = np.concatenate([np.transpose(nfi, (0, 2, 1)),
                         np.ones((G, 1, N), f32)], axis=1)   # [G, 17, N]
    nf = np.transpose(nf, (1, 0, 2)).reshape(DIN + 1, G * N)

    # adjb[g, mc, p, n] = keymask(m = mc*128+p source, n dest)
    adjT = ((np.transpose(adj, (0, 2, 1)) > 0) & (masks[:, :, None] > 0))
    adjb = adjT.astype(f32).reshape(G, MC, P, N)

    adjq = np.stack([(adj[g, qidx[g]] > 0) & (masks[g] > 0) for g in range(G)])
    adjq = adjq.astype(f32).reshape(G, MC, P).transpose(0, 2, 1)  # [G, P, MC]
    onehot = np.zeros((G, N), f32)
    onehot[np.arange(G), qidx] = 1.0
    onehot = onehot.reshape(G, MC, P).transpose(0, 2, 1)

    return {
        "nf": _bf16(nf),
        "adjb": _bf16(adjb),
        "adjq": _bf16(adjq),
        "oh": np.asarray(onehot, np.float32),
    }


def kernel(**inputs) -> np.ndarray:
    from concourse.bass_utils import run_bass_kernel_spmd

    nc = _build_nc()
    w = _prep_weights(inputs)
    in_maps = []
    for core in range(NCORES):
        m = _prep_core_inputs(inputs, core)
        m.update(w)
        in_maps.append(m)
    res = run_bass_kernel_spmd(nc, in_maps, list(range(NCORES)))
    out = np.concatenate([res.results[i]["out"][0] for i in range(NCORES)])
    return out.astype(np.float32).reshape(B, 1)
